# revision 1
# baseline (speedup 1.0000x reference)
"""Cross-modal selective-scan (ASSM) kernel for 8 TRN2 NeuronCores.

Sharding: one core per (batch, stream) pair: core = b*2 + s, s=0 rgb / s=1 e.
Each core computes the full forward for its stream (routing+gumbel of the
OTHER stream feeds C — cross-modal), the L=4096 selective scan over
(D=192, N=16) states, and the output layernorm. Outputs are gathered on host.

Per-core on-device layout: scan states live on partitions as 24 blocks of
(d8=8 channels x n=16 states) = 128 partitions; time runs along the free axis
so the hardware prefix-scan instruction (tensor_tensor_scan) computes
h_t = dA_t * h_{t-1} + dBu_t directly.
"""

import numpy as np

import concourse.bass as bass
import concourse.bacc as bacc
import concourse.mybir as mybir
import concourse.tile as tile
from concourse.bass_utils import run_bass_kernel_spmd

FP = mybir.dt.float32
OP = mybir.AluOpType

B, L, DM, N, R, T, H3 = 4, 4096, 192, 16, 12, 64, 64
P = 128
NBLK = DM // 8          # 24 d-blocks of 8 channels
LC = 1024               # scan chunk along L
NCH = L // LC           # 4
SUB = 512               # PSUM-friendly sub-chunk
NSUB = LC // SUB        # 2
LEPS = 1e-5

# packed-constant layout: name -> (col offset, rows, cols)
_CSHAPES = [
    ("w1T0", 128, 64), ("w1T1", 64, 64), ("b1r", 1, 64), ("w2T", 64, 64),
    ("b2r", 1, 64), ("PmRep", 64, 128), ("xpC0", 128, 128), ("xpC1", 64, 128),
    ("xpB0", 128, 128), ("xpB1", 64, 128), ("Mdt0", 128, 192),
    ("Mdt1", 64, 192), ("dtb0", 128, 1), ("dtb1", 64, 1),
    ("WdA0", 128, 16 * 128), ("WdA1", 64, 8 * 128), ("invA", 128, 1),
    ("S80", 128, 16 * 128), ("S81", 128, 8 * 64), ("ident", 128, 128),
    ("onesr", 1, 128), ("onc0", 128, 1), ("onc1", 64, 1), ("lng0", 128, 1),
    ("lng1", 64, 1), ("lnb0", 128, 1), ("lnb1", 64, 1), ("Dc0", 128, 1),
    ("Dc1", 64, 1),
]
CMAP = {}
_off = 0
for _nm, _r, _c in _CSHAPES:
    CMAP[_nm] = (_off, _r, _c)
    _off += _c
CTOT = _off


def build_program():
    nc = bacc.Bacc("TRN2", target_bir_lowering=False, debug=False)

    def din(name, shape):
        return nc.declare_dram_parameter(name, list(shape), FP, isOutput=False)

    # per-core data
    xsT0 = din("xsT0", (128, L))
    xsT1 = din("xsT1", (64, L))
    xoT0 = din("xoT0", (128, L))
    xoT1 = din("xoT1", (64, L))
    uo = din("uo", (L // 128, 128, T))
    # all small constants packed into one tensor -> one DMA, one semaphore
    cpack = din("cpack", (128, CTOT))

    yo0 = nc.declare_dram_parameter("yo0", [128, L], FP, isOutput=True)
    yo1 = nc.declare_dram_parameter("yo1", [64, L], FP, isOutput=True)

    AF = mybir.ActivationFunctionType

    with tile.TileContext(nc) as tc:
        with (
            tc.tile_pool(name="const", bufs=1) as cp,
            tc.tile_pool(name="xin", bufs=2) as xp,
            tc.tile_pool(name="proj", bufs=2) as pj,
            tc.tile_pool(name="route", bufs=2) as rt,
            tc.tile_pool(name="rsmall", bufs=5) as rs,
            tc.tile_pool(name="blk", bufs=2) as bk,
            tc.tile_pool(name="hpool", bufs=2) as hp,
            tc.tile_pool(name="ypool", bufs=1) as yp,
            tc.tile_pool(name="persist", bufs=1) as pe_,
            tc.tile_pool(name="ps_mm", bufs=2, space="PSUM") as ps_mm,
            tc.tile_pool(name="ps_pj", bufs=2, space="PSUM") as ps_pj,
            tc.tile_pool(name="ps_y", bufs=1, space="PSUM") as ps_y,
            tc.tile_pool(name="ps_rt", bufs=1, space="PSUM") as ps_rt,
        ):
            # ---- load all constants with one DMA ----
            cpk = cp.tile([128, CTOT], FP, tag="cpack")
            nc.sync.dma_start(cpk[:], cpack[:])

            def cv(name):
                off, rows, cols = CMAP[name]
                return cpk[0:rows, off:off + cols]

            c_w1T0 = cv("w1T0")
            c_w1T1 = cv("w1T1")
            c_b1r = cv("b1r")
            c_w2T = cv("w2T")
            c_b2r = cv("b2r")
            c_Pm = cv("PmRep")
            c_xpC0 = cv("xpC0")
            c_xpC1 = cv("xpC1")
            c_xpB0 = cv("xpB0")
            c_xpB1 = cv("xpB1")
            c_Mdt0 = cv("Mdt0")
            c_Mdt1 = cv("Mdt1")
            c_dtb0 = cv("dtb0")
            c_dtb1 = cv("dtb1")
            c_WdA0 = cv("WdA0")
            c_WdA1 = cv("WdA1")
            c_invA = cv("invA")
            c_S80 = cv("S80")
            c_S81 = cv("S81")
            c_id = cv("ident")
            c_1r = cv("onesr")
            c_on0 = cv("onc0")
            c_on1 = cv("onc1")
            c_lng0 = cv("lng0")
            c_lng1 = cv("lng1")
            c_lnb0 = cv("lnb0")
            c_lnb1 = cv("lnb1")
            c_Dc0 = cv("Dc0")
            c_Dc1 = cv("Dc1")

            hlast = pe_.tile([P, NBLK], FP)
            epsc = pe_.tile([128, 1], FP)
            nc.vector.memset(epsc[:], LEPS)
            # warm-touch cpack on each engine so its DMA wait is absorbed
            # once per engine (matmul LDWEIGHTS can hold only one sync wait)
            wtp = ps_rt.tile([1, 1], FP, tag="wt")
            nc.tensor.matmul(wtp[:], c_on0[:, 0:1], c_on0[:, 0:1],
                             start=True, stop=True)
            wts = pe_.tile([1, 2], FP)
            nc.vector.tensor_copy(wts[:, 0:1], c_on0[0:1, :])
            nc.scalar.copy(wts[:, 1:2], c_on0[0:1, :])

            for kc in range(NCH):
                ls = kc * LC

                xs0 = xp.tile([128, LC], FP, tag="xs0")
                xs1 = xp.tile([64, LC], FP, tag="xs1")
                xo0 = xp.tile([128, LC], FP, tag="xo0")
                xo1 = xp.tile([64, LC], FP, tag="xo1")
                nc.sync.dma_start(xs0[:], xsT0[:, ls:ls + LC])
                nc.sync.dma_start(xs1[:], xsT1[:, ls:ls + LC])
                nc.sync.dma_start(xo0[:], xoT0[:, ls:ls + LC])
                nc.sync.dma_start(xo1[:], xoT1[:, ls:ls + LC])

                # ---------- routing of the other stream -> one-hot OT ----------
                nt = LC // 128  # 8 l-tiles of 128
                OT = rt.tile([T, LC], FP, tag="OT")  # one-hot transposed
                hgel = [None] * nt
                # z1 + gelu (grouped so ACT sees one Gelu run)
                for i in range(nt):
                    z1 = ps_rt.tile([128, H3], FP, tag="psrt")
                    lo = ls + i * 128
                    nc.tensor.matmul(z1[:], xo0[:, i * 128:(i + 1) * 128],
                                     c_w1T0[:], start=True, stop=False)
                    nc.tensor.matmul(z1[:], xo1[:, i * 128:(i + 1) * 128],
                                     c_w1T1[:], start=False, stop=False)
                    nc.tensor.matmul(z1[:], c_1r[:], c_b1r[:],
                                     start=False, stop=True)
                    hg = rs.tile([128, H3], FP, tag="hgel", bufs=9)
                    nc.scalar.activation(hg[:], z1[:], AF.Gelu)
                    hgel[i] = hg
                # gumbel part: g = -log(-log u) ; do both Ln passes grouped
                ut = rt.tile([128, nt * T], FP, tag="ut", bufs=1)
                c0 = ls // 128
                nc.sync.dma_start(
                    ut[:].rearrange("p (c t) -> p c t", c=nt),
                    uo[c0:c0 + nt].rearrange("c p t -> p c t"))
                t2 = rt.tile([128, nt * T], FP, tag="t2")
                nc.scalar.activation(t2[:], ut[:], AF.Ln)
                nc.scalar.activation(t2[:], t2[:], AF.Ln, scale=-1.0)
                # z2 per tile, zg = z2 - t2, one-hot, transpose
                for i in range(nt):
                    hT = ps_rt.tile([H3, 128], FP, tag="psrt")
                    nc.tensor.transpose(hT[:], hgel[i][:], c_id[:])
                    hTs = rs.tile([H3, 128], FP, tag="hTs")
                    nc.vector.tensor_copy(hTs[:], hT[:])
                    z2 = ps_rt.tile([128, T], FP, tag="psrt")
                    nc.tensor.matmul(z2[:], hTs[:], c_w2T[:],
                                     start=True, stop=False)
                    nc.tensor.matmul(z2[:], c_1r[:], c_b2r[:],
                                     start=False, stop=True)
                    zg = rs.tile([128, T], FP, tag="zg")
                    nc.vector.tensor_tensor(
                        zg[:], z2[:], t2[:, i * T:(i + 1) * T], OP.subtract)
                    m8 = rs.tile([128, 8], FP, tag="m8")
                    nc.vector.max(m8[:], zg[:])
                    nc.vector.tensor_scalar(
                        zg[:], zg[:], m8[:, 0:1], None, OP.is_equal)
                    otp = ps_rt.tile([T, 128], FP, tag="psrt")
                    nc.tensor.transpose(otp[:], zg[:], c_id[:])
                    nc.vector.tensor_copy(OT[:, i * 128:(i + 1) * 128], otp[:])

                # ---------- projections: dt -> delta, w ; Brep ; Crep ----------
                dl0 = pj.tile([128, LC], FP, tag="dl0")
                dl1 = pj.tile([64, LC], FP, tag="dl1")
                Brep = pj.tile([128, LC], FP, tag="Brep")
                Crep = pj.tile([128, LC], FP, tag="Crep")
                for sb in range(NSUB):
                    sl = slice(sb * SUB, (sb + 1) * SUB)
                    dtp0 = ps_pj.tile([128, SUB], FP, tag="pspj")
                    nc.tensor.matmul(dtp0[:], c_Mdt0[:, 0:128],
                                     xs0[:, sl], start=True, stop=False)
                    nc.tensor.matmul(dtp0[:], c_Mdt1[:, 0:128],
                                     xs1[:, sl], start=False, stop=True)
                    sp_abs = pj.tile([128, SUB], FP, tag="sp_abs", bufs=1)
                    nc.scalar.activation(sp_abs[:], dtp0[:], AF.Abs,
                                         bias=c_dtb0[:])
                    nc.scalar.activation(sp_abs[:], sp_abs[:], AF.Exp,
                                         scale=-1.0)
                    nc.scalar.activation(sp_abs[:], sp_abs[:], AF.Ln,
                                         bias=c_on0[:])
                    nc.vector.tensor_scalar(dl0[:, sl], dtp0[:], c_dtb0[:],
                                            0.0, OP.add, OP.max)
                    nc.vector.tensor_tensor(dl0[:, sl], dl0[:, sl], sp_abs[:],
                                            OP.add)
                    dtp1 = ps_pj.tile([64, SUB], FP, tag="pspj")
                    nc.tensor.matmul(dtp1[:], c_Mdt0[:, 128:DM],
                                     xs0[:, sl], start=True, stop=False)
                    nc.tensor.matmul(dtp1[:], c_Mdt1[:, 128:DM],
                                     xs1[:, sl], start=False, stop=True)
                    sp_ab1 = pj.tile([64, SUB], FP, tag="sp_ab1", bufs=1)
                    nc.scalar.activation(sp_ab1[:], dtp1[:], AF.Abs,
                                         bias=c_dtb1[:])
                    nc.scalar.activation(sp_ab1[:], sp_ab1[:], AF.Exp,
                                         scale=-1.0)
                    nc.scalar.activation(sp_ab1[:], sp_ab1[:], AF.Ln,
                                         bias=c_on1[:])
                    nc.vector.tensor_scalar(dl1[:, sl], dtp1[:], c_dtb1[:],
                                            0.0, OP.add, OP.max)
                    nc.vector.tensor_tensor(dl1[:, sl], dl1[:, sl], sp_ab1[:],
                                            OP.add)
                    bp = ps_pj.tile([128, SUB], FP, tag="pspj")
                    nc.tensor.matmul(bp[:], c_xpB0[:], xs0[:, sl],
                                     start=True, stop=False)
                    nc.tensor.matmul(bp[:], c_xpB1[:], xs1[:, sl],
                                     start=False, stop=True)
                    nc.vector.tensor_copy(Brep[:, sl], bp[:])
                    cpp = ps_pj.tile([128, SUB], FP, tag="pspj")
                    for i in range(SUB // 128):
                        li = sb * SUB + i * 128
                        cl = slice(i * 128, (i + 1) * 128)
                        nc.tensor.matmul(cpp[:, cl], c_xpC0[:],
                                         xo0[:, li:li + 128],
                                         start=True, stop=False)
                        nc.tensor.matmul(cpp[:, cl], c_xpC1[:],
                                         xo1[:, li:li + 128],
                                         start=False, stop=False)
                        nc.tensor.matmul(cpp[:, cl], c_Pm[:],
                                         OT[:, li:li + 128],
                                         start=False, stop=True)
                    nc.vector.tensor_copy(Crep[:, sl], cpp[:])

                w0 = pj.tile([128, LC], FP, tag="w0")
                nc.vector.tensor_tensor(w0[:], dl0[:], xs0[:], OP.mult)
                w1_ = pj.tile([64, LC], FP, tag="w1_")
                nc.vector.tensor_tensor(w1_[:], dl1[:], xs1[:], OP.mult)

                # ---------- 24 scan blocks (16 full-tile, then 8) ----------
                def scan_block(j, yacc):
                    if j < 16:
                        dsrc, wsrc = dl0, w0
                        wa = c_WdA0[:, j * P:(j + 1) * P]
                        sc = c_S80[:, j * 128:(j + 1) * 128]
                        first, last = j == 0, j == 15
                    else:
                        dsrc, wsrc = dl1, w1_
                        wa = c_WdA1[:, (j - 16) * P:(j - 15) * P]
                        sc = c_S81[:, (j - 16) * 64:(j - 15) * 64]
                        first, last = j == 16, j == 23
                    dA = bk.tile([P, LC], FP, tag="dA")
                    dBu = bk.tile([P, LC], FP, tag="dBu")
                    for sb in range(NSUB):
                        sl = slice(sb * SUB, (sb + 1) * SUB)
                        aps = ps_mm.tile([P, SUB], FP, tag="psmm")
                        nc.tensor.matmul(aps[:], wa, dsrc[:, sl],
                                         start=True, stop=True)
                        nc.scalar.activation(dA[:, sl], aps[:], AF.Exp)
                        wps = ps_mm.tile([P, SUB], FP, tag="psmm")
                        nc.tensor.matmul(wps[:], wa, wsrc[:, sl],
                                         start=True, stop=True)
                        # wps = A[n]*w ; undo the A scale while folding in B
                        nc.vector.scalar_tensor_tensor(
                            dBu[:, sl], wps[:], c_invA[:], Brep[:, sl],
                            OP.mult, OP.mult)
                    h = hp.tile([P, LC], FP, tag="h")
                    init = 0.0 if kc == 0 else hlast[:, j:j + 1]
                    nc.vector.tensor_tensor_scan(
                        h[:], dA[:], dBu[:], init, OP.mult, OP.add)
                    if kc < NCH - 1:
                        nc.vector.tensor_copy(hlast[:, j:j + 1],
                                              h[:, LC - 1:LC])
                    ym = hp.tile([P, LC], FP, tag="ym")
                    nc.vector.tensor_tensor(ym[:], h[:], Crep[:], OP.mult)
                    for sb in range(NSUB):
                        sl = slice(sb * SUB, (sb + 1) * SUB)
                        nc.tensor.matmul(yacc[sb][:], sc, ym[:, sl],
                                         start=first, stop=last)

                yac0 = [ps_y.tile([128, SUB], FP, tag=f"yps{sb}",
                                  name=f"yac0_{kc}_{sb}")
                        for sb in range(NSUB)]
                for j in range(16):
                    scan_block(j, yac0)
                yD0 = yp.tile([128, LC], FP, tag="yD0")
                for sb in range(NSUB):
                    sl = slice(sb * SUB, (sb + 1) * SUB)
                    nc.vector.scalar_tensor_tensor(
                        yD0[:, sl], xs0[:, sl], c_Dc0[:], yac0[sb][:],
                        OP.mult, OP.add)
                yac1 = [ps_y.tile([64, SUB], FP, tag=f"yps{sb}",
                                  name=f"yac1_{kc}_{sb}")
                        for sb in range(NSUB)]
                for j in range(16, 24):
                    scan_block(j, yac1)
                yD1 = yp.tile([64, LC], FP, tag="yD1")
                for sb in range(NSUB):
                    sl = slice(sb * SUB, (sb + 1) * SUB)
                    nc.vector.scalar_tensor_tensor(
                        yD1[:, sl], xs1[:, sl], c_Dc1[:], yac1[sb][:],
                        OP.mult, OP.add)
                ysq0 = yp.tile([128, LC], FP, tag="ysq0")
                nc.scalar.activation(ysq0[:], yD0[:], AF.Square)
                ysq1 = yp.tile([64, LC], FP, tag="ysq1")
                nc.scalar.activation(ysq1[:], yD1[:], AF.Square)

                s1row = rt.tile([1, LC], FP, tag="s1row", bufs=1)
                s2row = rt.tile([1, LC], FP, tag="s2row", bufs=1)
                for sb in range(NSUB):
                    sl = slice(sb * SUB, (sb + 1) * SUB)
                    s1p = ps_pj.tile([1, SUB], FP, tag="pspj")
                    nc.tensor.matmul(s1p[:], c_on0[:], yD0[:, sl],
                                     start=True, stop=False)
                    nc.tensor.matmul(s1p[:], c_on1[:], yD1[:, sl],
                                     start=False, stop=True)
                    nc.vector.tensor_copy(s1row[:, sl], s1p[:])
                    s2p = ps_pj.tile([1, SUB], FP, tag="pspj")
                    nc.tensor.matmul(s2p[:], c_on0[:], ysq0[:, sl],
                                     start=True, stop=False)
                    nc.tensor.matmul(s2p[:], c_on1[:], ysq1[:, sl],
                                     start=False, stop=True)
                    nc.vector.tensor_copy(s2row[:, sl], s2p[:])
                # reshape rows to [128, LC/128] for fast elementwise stats
                sm = rt.tile([128, 2 * (LC // 128)], FP, tag="sm")
                nw = LC // 128
                nc.sync.dma_start(sm[:, 0:nw], s1row[:])
                nc.sync.dma_start(sm[:, nw:2 * nw], s2row[:])
                mu = rt.tile([128, nw], FP, tag="mu")
                nc.vector.tensor_scalar(mu[:], sm[:, 0:nw], 1.0 / DM, None,
                                        OP.mult)
                musq = rt.tile([128, nw], FP, tag="musq")
                nc.scalar.activation(musq[:], mu[:], AF.Square)
                var = rt.tile([128, nw], FP, tag="var")
                nc.vector.scalar_tensor_tensor(
                    var[:], sm[:, nw:2 * nw], 1.0 / DM, musq[:],
                    OP.mult, OP.subtract)
                sd = rt.tile([128, nw], FP, tag="sd")
                nc.scalar.activation(sd[:], var[:], AF.Sqrt, bias=epsc[:])
                inv = rt.tile([128, nw], FP, tag="inv")
                nc.vector.reciprocal(inv[:], sd[:])
                mrow = rt.tile([1, LC], FP, tag="mrow")
                nc.sync.dma_start(mrow[:], mu[:])
                irow = rt.tile([1, LC], FP, tag="irow")
                nc.sync.dma_start(irow[:], inv[:])
                # broadcast mu/inv across partitions via k=1 matmuls
                for sb in range(NSUB):
                    sl = slice(sb * SUB, (sb + 1) * SUB)
                    mb = ps_pj.tile([128, SUB], FP, tag="pspj")
                    nc.tensor.matmul(mb[:], c_1r[:], mrow[0:1, sl],
                                     start=True, stop=True)
                    nc.vector.tensor_tensor(yD0[:, sl], yD0[:, sl], mb[:],
                                            OP.subtract)
                    nc.vector.tensor_tensor(yD1[:, sl], yD1[:, sl],
                                            mb[0:64, :], OP.subtract)
                    ib = ps_pj.tile([128, SUB], FP, tag="pspj")
                    nc.tensor.matmul(ib[:], c_1r[:], irow[0:1, sl],
                                     start=True, stop=True)
                    nc.vector.tensor_tensor(yD0[:, sl], yD0[:, sl], ib[:],
                                            OP.mult)
                    nc.vector.tensor_tensor(yD1[:, sl], yD1[:, sl],
                                            ib[0:64, :], OP.mult)
                nc.vector.tensor_scalar(yD0[:], yD0[:], c_lng0[:], c_lnb0[:],
                                        OP.mult, OP.add)
                nc.sync.dma_start(yo0[:, ls:ls + LC], yD0[:])
                nc.vector.tensor_scalar(yD1[:], yD1[:], c_lng1[:], c_lnb1[:],
                                        OP.mult, OP.add)
                nc.sync.dma_start(yo1[:, ls:ls + LC], yD1[:])

    nc.compile()
    return nc


_PROG = None


def _get_prog():
    global _PROG
    if _PROG is None:
        _PROG = build_program()
    return _PROG


def _make_in_maps(inputs):
    f32 = lambda a: np.ascontiguousarray(np.asarray(a, dtype=np.float32))
    x = {0: f32(inputs["x_rgb"]), 1: f32(inputs["x_e"])}
    u = {0: f32(inputs["u_rgb"]), 1: f32(inputs["u_e"])}
    rw1 = {0: f32(inputs["route_rgb_w1"]), 1: f32(inputs["route_e_w1"])}
    rb1 = {0: f32(inputs["route_rgb_b1"]), 1: f32(inputs["route_e_b1"])}
    rw2 = {0: f32(inputs["route_rgb_w2"]), 1: f32(inputs["route_e_w2"])}
    rb2 = {0: f32(inputs["route_rgb_b2"]), 1: f32(inputs["route_e_b2"])}
    emb = {0: f32(inputs["emb_rgb"]), 1: f32(inputs["emb_e"])}
    tok = {0: f32(inputs["token_rgb_w"]), 1: f32(inputs["token_e_w"])}
    xproj = {0: f32(inputs["xproj_rgb"]), 1: f32(inputs["xproj_e"])}
    dtw = {0: f32(inputs["dtw_rgb"]), 1: f32(inputs["dtw_e"])}
    dtb = {0: f32(inputs["dtb_rgb"]), 1: f32(inputs["dtb_e"])}
    Alog = {0: f32(inputs["Alog_rgb"]), 1: f32(inputs["Alog_e"])}
    Dsk = {0: f32(inputs["D_rgb"]), 1: f32(inputs["D_e"])}
    lng = {0: f32(inputs["ln1_g"]), 1: f32(inputs["ln2_g"])}
    lnb = {0: f32(inputs["ln1_b"]), 1: f32(inputs["ln2_b"])}

    nmap = np.arange(P) % 16   # p -> n
    dmap = np.arange(P) // 16  # p -> d8
    ident = np.eye(P, dtype=np.float32)
    onesr = np.ones((1, P), np.float32)
    # per-block scatter matrices: block j of tile0 covers d-rows 8j..8j+8
    S80 = np.zeros((16, P, 128), np.float32)
    for j in range(16):
        S80[j, np.arange(P), 8 * j + dmap] = 1.0
    S81 = np.zeros((8, P, 64), np.float32)
    for j in range(8):
        S81[j, np.arange(P), 8 * j + dmap] = 1.0

    in_maps = []
    for c in range(8):
        b, s = divmod(c, 2)
        o = 1 - s
        xsT = x[s][b].T.copy()          # [192, L]
        xoT = x[o][b].T.copy()
        A = -np.exp(Alog[s])            # [DM, N]
        assert np.allclose(A, A[0:1, :], atol=0), "A must be d-independent"
        Arow = A[0]                     # [N]
        WdA0 = np.zeros((16, 128, P), np.float32)
        for j in range(16):
            WdA0[j, 8 * j + dmap, np.arange(P)] = Arow[nmap]
        WdA1 = np.zeros((8, 64, P), np.float32)
        for j in range(8):
            WdA1[j, 8 * j + dmap, np.arange(P)] = Arow[nmap]
        Pm = emb[o] @ tok[o]            # [T, N]
        PmRep = np.ascontiguousarray(Pm[:, nmap])          # [T, P]
        CrepT = np.ascontiguousarray(xproj[o][R + N:R + 2 * N][nmap].T)  # [DM,P]
        BrepT = np.ascontiguousarray(xproj[s][R:R + N][nmap].T)
        Mdt = (dtw[s] @ xproj[s][:R]).T.copy()             # [DM(k), DM(out)]
        consts = {
            "w1T0": rw1[o].T[:128], "w1T1": rw1[o].T[128:],
            "b1r": rb1[o][None, :], "w2T": rw2[o].T, "b2r": rb2[o][None, :],
            "PmRep": PmRep, "xpC0": CrepT[:128], "xpC1": CrepT[128:],
            "xpB0": BrepT[:128], "xpB1": BrepT[128:],
            "Mdt0": Mdt[:128], "Mdt1": Mdt[128:],
            "dtb0": dtb[s][:128, None], "dtb1": dtb[s][128:, None],
            "WdA0": np.transpose(WdA0, (1, 0, 2)).reshape(128, 16 * P),
            "WdA1": np.transpose(WdA1, (1, 0, 2)).reshape(64, 8 * P),
            "invA": (1.0 / Arow[nmap])[:, None],
            "S80": np.transpose(S80, (1, 0, 2)).reshape(P, 16 * 128),
            "S81": np.transpose(S81, (1, 0, 2)).reshape(P, 8 * 64),
            "ident": ident, "onesr": onesr,
            "onc0": np.ones((128, 1), np.float32),
            "onc1": np.ones((64, 1), np.float32),
            "lng0": lng[s][:128, None], "lng1": lng[s][128:, None],
            "lnb0": lnb[s][:128, None], "lnb1": lnb[s][128:, None],
            "Dc0": Dsk[s][:128, None], "Dc1": Dsk[s][128:, None],
        }
        cpk = np.zeros((128, CTOT), np.float32)
        for nm, (off, rows, cols) in CMAP.items():
            a = np.asarray(consts[nm], dtype=np.float32)
            assert a.shape == (rows, cols), (nm, a.shape, rows, cols)
            cpk[:rows, off:off + cols] = a
        m = {
            "xsT0": xsT[:128], "xsT1": xsT[128:],
            "xoT0": xoT[:128], "xoT1": xoT[128:],
            "uo": u[o][b].reshape(L // 128, 128, T).copy(),
            "cpack": cpk,
        }
        in_maps.append({k: np.ascontiguousarray(v, dtype=np.float32)
                        for k, v in m.items()})
    return in_maps


def run(inputs, trace=False):
    nc = _get_prog()
    in_maps = _make_in_maps(inputs)
    res = run_bass_kernel_spmd(nc, in_maps, list(range(8)), trace=trace)
    out = np.zeros((2, B, DM, L), np.float32)
    for c in range(8):
        b, s = divmod(c, 2)
        out[s, b, :128] = res.results[c]["yo0"]
        out[s, b, 128:] = res.results[c]["yo1"]
    return out, res


def kernel(**inputs):
    out, _ = run(inputs, trace=False)
    return out



# revision 6
# speedup vs baseline: 1.4075x; 1.4075x over previous
"""Cross-modal selective-scan (ASSM) kernel for 8 TRN2 NeuronCores.

Sharding: one core per (batch, stream) pair: core = b*2 + s, s=0 rgb / s=1 e.
Each core computes the full forward for its stream (routing+gumbel of the
OTHER stream feeds C — cross-modal), the L=4096 selective scan over
(D=192, N=16) states, and the output layernorm. Outputs are gathered on host.

v2 layout highlights:
 - matmuls run in bf16 (x-projections / routing; weights exact or ~4e-3) and
   fp32r (scan path, ~1.2e-4) instead of fp32 -> ~4x PE throughput.
 - dA-arg and dBu-arg share one fused matmul per (block, sub): rhs = [dl | w].
 - gumbel noise -ln(-ln u) (+ b2 routing bias) is precomputed on host.
 - elementwise work is split Vector/GpSimd/Scalar: GpSimd cannot touch PSUM,
   so PSUM-reading ops (dBu, LN apply) stay on Vector; ym/w-build go to Pool.
"""

import numpy as np
import ml_dtypes

import concourse.bass as bass
import concourse.bacc as bacc
import concourse.mybir as mybir
import concourse.tile as tile
from concourse.bass_utils import run_bass_kernel_spmd

FP = mybir.dt.float32
FPR = mybir.dt.float32r
BF = mybir.dt.bfloat16
OP = mybir.AluOpType
AF = mybir.ActivationFunctionType

B, L, DM, N, R, T, H3 = 4, 4096, 192, 16, 12, 64, 64
P = 128
LC = 1024               # chunk along L
NCH = L // LC           # 4
SUB = 512
LEPS = 1e-5

# ym placement: True -> GpSimd (Pool), False -> Vector. Balance the engines.
YM_ON_POOL = [True, True, False] * 8  # 16 pool, 8 vector

# ---- packed-constant layouts: name -> (col offset, rows, cols) ----
def _pack(shapes):
    m, off = {}, 0
    for nm, r, c in shapes:
        m[nm] = (off, r, c)
        off += c
    return m, off

_CB_SHAPES = [
    ("w1T0", 128, 64), ("w1T1", 64, 64), ("w2T", 64, 64), ("PmRep", 64, 128),
    ("xpC0", 128, 128), ("xpC1", 64, 128), ("xpB0", 128, 128),
    ("xpB1", 64, 128), ("Mdt0", 128, 192), ("Mdt1", 64, 192),
]
CBMAP, CBTOT = _pack(_CB_SHAPES)

_CR_SHAPES = [
    ("WdA0", 128, 16 * 128), ("WdA1", 64, 8 * 128), ("S80", 128, 16 * 128),
    ("S81", 128, 8 * 64), ("onc0", 128, 1), ("onc1", 64, 1),
]
CRMAP, CRTOT = _pack(_CR_SHAPES)

_CF_SHAPES = [
    ("identF", 128, 128), ("b1c", 64, 1), ("dtb0", 128, 1), ("dtb1", 64, 1),
    ("invA", 128, 1), ("Dc0", 128, 1), ("Dc1", 64, 1), ("lnb0", 128, 1),
    ("lnb1", 64, 1), ("lngr0", 1, 128), ("lngr1", 1, 64),
]
CFMAP, CFTOT = _pack(_CF_SHAPES)


def build_program():
    nc = bacc.Bacc("TRN2", target_bir_lowering=False, debug=False)

    xsT0 = nc.declare_dram_parameter("xsT0", [128, L], BF, isOutput=False)
    xsT1 = nc.declare_dram_parameter("xsT1", [64, L], BF, isOutput=False)
    xoT0 = nc.declare_dram_parameter("xoT0", [128, L], BF, isOutput=False)
    xoT1 = nc.declare_dram_parameter("xoT1", [64, L], BF, isOutput=False)
    gq = nc.declare_dram_parameter("gq", [L // 128, 128, T], FP, isOutput=False)
    cpb = nc.declare_dram_parameter("cpb", [128, CBTOT], BF, isOutput=False)
    cpr = nc.declare_dram_parameter("cpr", [128, CRTOT], FPR, isOutput=False)
    cpf = nc.declare_dram_parameter("cpf", [128, CFTOT], FP, isOutput=False)
    yo0 = nc.declare_dram_parameter("yo0", [128, L], FP, isOutput=True)
    yo1 = nc.declare_dram_parameter("yo1", [64, L], FP, isOutput=True)

    with tile.TileContext(nc) as tc:
        with (
            tc.tile_pool(name="const", bufs=1) as cp,
            tc.tile_pool(name="xin", bufs=2) as xp,
            tc.tile_pool(name="dwp", bufs=2) as dwp,
            tc.tile_pool(name="proj", bufs=2) as pj,
            tc.tile_pool(name="route", bufs=2) as rt,
            tc.tile_pool(name="spool", bufs=1) as sp_,
            tc.tile_pool(name="blk", bufs=2) as bk,
            tc.tile_pool(name="hpool", bufs=2) as hp,
            tc.tile_pool(name="ypool", bufs=1) as yp,
            tc.tile_pool(name="rows", bufs=1) as rw,
            tc.tile_pool(name="persist", bufs=1) as pe_,
            tc.tile_pool(name="ps_mm", bufs=2, space="PSUM") as ps_mm,
            tc.tile_pool(name="ps_y", bufs=1, space="PSUM") as ps_y,
            tc.tile_pool(name="ps_s", bufs=2, space="PSUM") as ps_s,
        ):
            cbk = cp.tile([128, CBTOT], BF, tag="cpb")
            nc.sync.dma_start(cbk[:], cpb[:])
            crk = cp.tile([128, CRTOT], FPR, tag="cpr")
            nc.sync.dma_start(crk[:], cpr[:])
            cfk = cp.tile([128, CFTOT], FP, tag="cpf")
            nc.sync.dma_start(cfk[:], cpf[:])

            def cb(name):
                off, r, c = CBMAP[name]
                return cbk[0:r, off:off + c]

            def cr(name):
                off, r, c = CRMAP[name]
                return crk[0:r, off:off + c]

            def cf(name):
                off, r, c = CFMAP[name]
                return cfk[0:r, off:off + c]

            def mm512(out, lhsT, rhs, start, stop):
                # ISA caps the moving operand at 512 elements per matmul
                n = rhs.shape[-1]
                for q in range(0, n, 512):
                    e = min(q + 512, n)
                    nc.tensor.matmul(out[:, q:e], lhsT, rhs[:, q:e],
                                     start=start, stop=stop)

            hlast = pe_.tile([P, 24], FP)
            epsc = pe_.tile([128, 1], FP)
            nc.vector.memset(epsc[:], LEPS)

            for kc in range(NCH):
                ls = kc * LC
                c0 = ls // 128

                xs0 = xp.tile([128, LC], BF, tag="xs0")
                xs1 = xp.tile([64, LC], BF, tag="xs1")
                xo0 = xp.tile([128, LC], BF, tag="xo0")
                xo1 = xp.tile([64, LC], BF, tag="xo1")
                gt = xp.tile([128, (LC // 128) * T], FP, tag="gt")
                nc.sync.dma_start(xs0[:], xsT0[:, ls:ls + LC])
                nc.sync.dma_start(xs1[:], xsT1[:, ls:ls + LC])
                nc.sync.dma_start(xo0[:], xoT0[:, ls:ls + LC])
                nc.sync.dma_start(xo1[:], xoT1[:, ls:ls + LC])
                nt = LC // 128  # 8 token tiles per chunk
                nc.sync.dma_start(
                    gt[:].rearrange("p (c t) -> p c t", c=nt),
                    gq[c0:c0 + nt].rearrange("c p t -> p c t"))

                # ---------- routing of the other stream -> one-hot OT ----------
                zt = ps_mm.tile([128, LC], FP, tag="mm", name=f"zt{kc}")
                mm512(zt[0:H3, :], cb("w1T0"), xo0[:], True, False)
                mm512(zt[0:H3, :], cb("w1T1"), xo1[:], False, True)
                hg = rt.tile([H3, LC], BF, tag="hg")
                nc.scalar.activation(hg[:], zt[0:H3, :], AF.Gelu,
                                     bias=cf("b1c"))
                z2 = ps_s.tile([128, nt * T], FP, tag="s", name=f"z2{kc}")
                for i in range(nt):
                    nc.tensor.matmul(z2[:, i * T:(i + 1) * T],
                                     hg[:, i * 128:(i + 1) * 128], cb("w2T"),
                                     start=True, stop=True)
                zg = rt.tile([128, nt * T], FP, tag="zg")
                nc.vector.tensor_tensor(zg[:], z2[:], gt[:], OP.add)
                oh = rt.tile([128, nt * T], FP, tag="oh")
                for i in range(nt):
                    sl = slice(i * T, (i + 1) * T)
                    m8 = rt.tile([128, 8], FP, tag="m8", bufs=3)
                    nc.vector.max(m8[:], zg[:, sl])
                    nc.vector.tensor_scalar(oh[:, sl], zg[:, sl], m8[:, 0:1],
                                            None, OP.is_equal)
                tp = ps_mm.tile([128, LC], FP, tag="mm", name=f"tp{kc}")
                for i in range(nt):
                    nc.tensor.transpose(tp[0:T, i * 128:(i + 1) * 128],
                                        oh[:, i * T:(i + 1) * T], cf("identF"))
                OT = rt.tile([T, LC], BF, tag="OT")
                nc.scalar.copy(OT[:], tp[0:T, :])

                # ---------- projections ----------
                dw0 = dwp.tile([128, 2 * LC], FPR, tag="dw0")
                dw1 = dwp.tile([64, 2 * LC], FPR, tag="dw1")
                # views: [p, sub, half(dl|w), SUB]
                dv0 = dw0[:].rearrange("p (s h k) -> p s h k", s=2, h=2)
                dv1 = dw1[:].rearrange("p (s h k) -> p s h k", s=2, h=2)

                dtp0 = ps_mm.tile([128, LC], FP, tag="mm", name=f"dt0{kc}")
                mm512(dtp0[:], cb("Mdt0")[:, 0:128], xs0[:], True, False)
                mm512(dtp0[:], cb("Mdt1")[:, 0:128], xs1[:], False, True)
                dtp1 = ps_mm.tile([64, LC], FP, tag="mm", name=f"dt1{kc}")
                mm512(dtp1[:], cb("Mdt0")[:, 128:DM], xs0[:], True, False)
                mm512(dtp1[:], cb("Mdt1")[:, 128:DM], xs1[:], False, True)
                # softplus(dt + dtb) = relu(x) + ln(1 + exp(-|x|))
                sp0 = sp_.tile([128, LC], FP, tag="sp0")
                sp1 = sp_.tile([64, LC], FP, tag="sp1")
                dtv0 = dtp0[:].rearrange("p (s k) -> p s k", s=2)
                dtv1 = dtp1[:].rearrange("p (s k) -> p s k", s=2)
                nc.scalar.activation(sp0[:], dtp0[:], AF.Abs, bias=cf("dtb0"))
                nc.scalar.activation(sp1[:], dtp1[:], AF.Abs, bias=cf("dtb1"))
                nc.scalar.activation(sp0[:], sp0[:], AF.Exp, scale=-1.0)
                nc.scalar.activation(sp1[:], sp1[:], AF.Exp, scale=-1.0)
                nc.scalar.activation(sp0[:], sp0[:], AF.Ln, bias=1.0)
                nc.scalar.activation(sp1[:], sp1[:], AF.Ln, bias=1.0)
                nc.scalar.activation(dv0[:, :, 0, :], dtv0[:], AF.Relu,
                                     bias=cf("dtb0"))
                nc.scalar.activation(dv1[:, :, 0, :], dtv1[:], AF.Relu,
                                     bias=cf("dtb1"))
                sv0 = sp0[:].rearrange("p (s k) -> p s k", s=2)
                sv1 = sp1[:].rearrange("p (s k) -> p s k", s=2)
                nc.gpsimd.tensor_tensor(dv0[:, :, 0, :],
                                        dv0[:, :, 0, :].bitcast(FP),
                                        sv0[:], OP.add)
                nc.gpsimd.tensor_tensor(dv1[:, :, 0, :],
                                        dv1[:, :, 0, :].bitcast(FP),
                                        sv1[:], OP.add)
                # w = dl * x  (into the | w | halves)
                xv0 = xs0[:].rearrange("p (s k) -> p s k", s=2)
                xv1 = xs1[:].rearrange("p (s k) -> p s k", s=2)
                nc.gpsimd.tensor_tensor(dv0[:, :, 1, :],
                                        dv0[:, :, 0, :].bitcast(FP),
                                        xv0[:], OP.mult)
                nc.gpsimd.tensor_tensor(dv1[:, :, 1, :],
                                        dv1[:, :, 0, :].bitcast(FP),
                                        xv1[:], OP.mult)

                bp = ps_mm.tile([128, LC], FP, tag="mm", name=f"bp{kc}")
                mm512(bp[:], cb("xpB0"), xs0[:], True, False)
                mm512(bp[:], cb("xpB1"), xs1[:], False, True)
                BrepI = pj.tile([128, LC], FP, tag="BrepI")
                nc.scalar.activation(BrepI[:], bp[:], AF.Copy,
                                     scale=cf("invA"))
                cpp = ps_mm.tile([128, LC], FP, tag="mm", name=f"cp{kc}")
                mm512(cpp[:], cb("xpC0"), xo0[:], True, False)
                mm512(cpp[:], cb("xpC1"), xo1[:], False, False)
                mm512(cpp[:], cb("PmRep"), OT[:], False, True)
                Crep = pj.tile([128, LC], FP, tag="Crep")
                nc.scalar.copy(Crep[:], cpp[:])

                # ---------- 24 scan blocks ----------
                yac0 = ps_y.tile([128, LC], FP, tag="y", name=f"y0_{kc}")
                yac1 = None
                yD0 = yD1 = None

                for j in range(24):
                    if j < 16:
                        dwt = dw0
                        wa = cr("WdA0")[:, j * P:(j + 1) * P]
                        sc = cr("S80")[:, j * 128:(j + 1) * 128]
                        yac = yac0
                        first, last = j == 0, j == 15
                    else:
                        dwt = dw1
                        wa = cr("WdA1")[:, (j - 16) * P:(j - 15) * P]
                        sc = cr("S81")[:, (j - 16) * 64:(j - 15) * 64]
                        yac = yac1
                        first, last = j == 16, j == 23
                    dA = bk.tile([P, LC], FP, tag="dA")
                    dBu = bk.tile([P, LC], FP, tag="dBu")
                    for sb in range(2):
                        mmp = ps_mm.tile([128, LC], FP, tag="mm",
                                         name=f"mm{kc}_{j}_{sb}")
                        mm512(mmp[:], wa, dwt[:, sb * LC:(sb + 1) * LC],
                              True, True)
                        ssl = slice(sb * SUB, (sb + 1) * SUB)
                        nc.scalar.activation(dA[:, ssl], mmp[:, 0:SUB], AF.Exp)
                        nc.vector.tensor_tensor(dBu[:, ssl], mmp[:, SUB:LC],
                                                BrepI[:, ssl], OP.mult)
                    h = hp.tile([P, LC], FP, tag="h")
                    init = 0.0 if kc == 0 else hlast[:, j:j + 1]
                    nc.vector.tensor_tensor_scan(h[:], dA[:], dBu[:], init,
                                                 OP.mult, OP.add)
                    if kc < NCH - 1:
                        nc.vector.tensor_copy(hlast[:, j:j + 1],
                                              h[:, LC - 1:LC])
                    ym = hp.tile([P, LC], FPR, tag="ym")
                    eng = nc.gpsimd if YM_ON_POOL[j] else nc.vector
                    eng.tensor_tensor(ym[:], h[:], Crep[:], OP.mult)
                    mm512(yac[0:P if j < 16 else 64, :], sc, ym[:],
                          first, last)
                    if j == 15:
                        yD0 = yp.tile([128, LC], FPR, tag="yD0")
                        nc.vector.scalar_tensor_tensor(
                            yD0[:], xs0[:], cf("Dc0"), yac0[:],
                            OP.mult, OP.add)
                        yac1 = ps_y.tile([64, LC], FP, tag="y",
                                         name=f"y1_{kc}")

                yD1 = yp.tile([64, LC], FPR, tag="yD1")
                nc.vector.scalar_tensor_tensor(
                    yD1[:], xs1[:], cf("Dc1"), yac1[:], OP.mult, OP.add)
                ysq0 = yp.tile([128, LC], FPR, tag="ysq0")
                nc.scalar.activation(ysq0[:], yD0[:].bitcast(FP), AF.Square)
                ysq1 = yp.tile([64, LC], FPR, tag="ysq1")
                nc.scalar.activation(ysq1[:], yD1[:].bitcast(FP), AF.Square)

                # ---------- layernorm ----------
                s1p = ps_mm.tile([128, LC], FP, tag="mm", name=f"s1{kc}")
                mm512(s1p[0:1, :], cr("onc0"), yD0[:], True, False)
                mm512(s1p[0:1, :], cr("onc1"), yD1[:], False, True)
                s2p = ps_mm.tile([128, LC], FP, tag="mm", name=f"s2{kc}")
                mm512(s2p[0:1, :], cr("onc0"), ysq0[:], True, False)
                mm512(s2p[0:1, :], cr("onc1"), ysq1[:], False, True)
                s1row = rw.tile([1, LC], FP, tag="s1row")
                s2row = rw.tile([1, LC], FP, tag="s2row")
                nc.scalar.copy(s1row[:], s1p[0:1, :])
                nc.scalar.copy(s2row[:], s2p[0:1, :])
                nw = LC // 128
                sm = rw.tile([128, 2 * nw], FP, tag="sm")
                nc.sync.dma_start(sm[:, 0:nw], s1row[:])
                nc.sync.dma_start(sm[:, nw:2 * nw], s2row[:])
                mu = rw.tile([128, nw], FP, tag="mu")
                nc.vector.tensor_scalar(mu[:], sm[:, 0:nw], 1.0 / DM, None,
                                        OP.mult)
                musq = rw.tile([128, nw], FP, tag="musq")
                nc.scalar.activation(musq[:], mu[:], AF.Square)
                var = rw.tile([128, nw], FP, tag="var")
                nc.vector.scalar_tensor_tensor(
                    var[:], sm[:, nw:2 * nw], 1.0 / DM, musq[:],
                    OP.mult, OP.subtract)
                sd = rw.tile([128, nw], FP, tag="sd")
                nc.scalar.activation(sd[:], var[:], AF.Sqrt, bias=epsc[:])
                inv = rw.tile([128, nw], FP, tag="inv")
                nc.vector.reciprocal(inv[:], sd[:])
                mui = rw.tile([128, nw], FP, tag="mui")
                nc.vector.tensor_tensor(mui[:], mu[:], inv[:], OP.mult)
                irow = rw.tile([1, LC], FP, tag="irow")
                nc.sync.dma_start(irow[:], inv[:])
                mirow = rw.tile([1, LC], FP, tag="mirow")
                nc.sync.dma_start(mirow[:], mui[:])

                # broadcast g*inv and g*mu*inv via k=1 fp32 matmuls (512-col)
                ib0 = ps_mm.tile([128, LC], FP, tag="mm", name=f"ib0{kc}")
                mi0 = ps_mm.tile([128, LC], FP, tag="mm", name=f"mi0{kc}")
                for sb in range(2):
                    ssl = slice(sb * SUB, (sb + 1) * SUB)
                    nc.tensor.matmul(ib0[:, ssl], cf("lngr0"), irow[:, ssl],
                                     start=True, stop=True)
                    nc.tensor.matmul(mi0[:, ssl], cf("lngr0"), mirow[:, ssl],
                                     start=True, stop=True)
                yo0t = yp.tile([128, LC], FP, tag="yo0t")
                nc.vector.tensor_tensor(yo0t[:], yD0[:].bitcast(FP), ib0[:],
                                        OP.mult)
                nc.vector.scalar_tensor_tensor(
                    yo0t[:], yo0t[:], cf("lnb0"), mi0[:], OP.add, OP.subtract)
                nc.sync.dma_start(yo0[:, ls:ls + LC], yo0t[:])

                ib1 = ps_mm.tile([64, LC], FP, tag="mm", name=f"ib1{kc}")
                mi1 = ps_mm.tile([64, LC], FP, tag="mm", name=f"mi1{kc}")
                for sb in range(2):
                    ssl = slice(sb * SUB, (sb + 1) * SUB)
                    nc.tensor.matmul(ib1[:, ssl], cf("lngr1"), irow[:, ssl],
                                     start=True, stop=True)
                    nc.tensor.matmul(mi1[:, ssl], cf("lngr1"), mirow[:, ssl],
                                     start=True, stop=True)
                yo1t = yp.tile([64, LC], FP, tag="yo1t")
                nc.vector.tensor_tensor(yo1t[:], yD1[:].bitcast(FP), ib1[:],
                                        OP.mult)
                nc.vector.scalar_tensor_tensor(
                    yo1t[:], yo1t[:], cf("lnb1"), mi1[:], OP.add, OP.subtract)
                nc.sync.dma_start(yo1[:, ls:ls + LC], yo1t[:])

    nc.compile()
    return nc


_PROG = None


def _get_prog():
    global _PROG
    if _PROG is None:
        _PROG = build_program()
    return _PROG


def _make_in_maps(inputs):
    f32 = lambda a: np.ascontiguousarray(np.asarray(a, dtype=np.float32))
    bf16 = lambda a: np.ascontiguousarray(
        np.asarray(np.asarray(a, dtype=np.float32), dtype=ml_dtypes.bfloat16))
    x = {0: f32(inputs["x_rgb"]), 1: f32(inputs["x_e"])}
    u = {0: f32(inputs["u_rgb"]), 1: f32(inputs["u_e"])}
    rw1 = {0: f32(inputs["route_rgb_w1"]), 1: f32(inputs["route_e_w1"])}
    rb1 = {0: f32(inputs["route_rgb_b1"]), 1: f32(inputs["route_e_b1"])}
    rw2 = {0: f32(inputs["route_rgb_w2"]), 1: f32(inputs["route_e_w2"])}
    rb2 = {0: f32(inputs["route_rgb_b2"]), 1: f32(inputs["route_e_b2"])}
    emb = {0: f32(inputs["emb_rgb"]), 1: f32(inputs["emb_e"])}
    tok = {0: f32(inputs["token_rgb_w"]), 1: f32(inputs["token_e_w"])}
    xproj = {0: f32(inputs["xproj_rgb"]), 1: f32(inputs["xproj_e"])}
    dtw = {0: f32(inputs["dtw_rgb"]), 1: f32(inputs["dtw_e"])}
    dtb = {0: f32(inputs["dtb_rgb"]), 1: f32(inputs["dtb_e"])}
    Alog = {0: f32(inputs["Alog_rgb"]), 1: f32(inputs["Alog_e"])}
    Dsk = {0: f32(inputs["D_rgb"]), 1: f32(inputs["D_e"])}
    lng = {0: f32(inputs["ln1_g"]), 1: f32(inputs["ln2_g"])}
    lnb = {0: f32(inputs["ln1_b"]), 1: f32(inputs["ln2_b"])}

    nmap = np.arange(P) % 16   # p -> n
    dmap = np.arange(P) // 16  # p -> d8

    in_maps = []
    for c in range(8):
        b, s = divmod(c, 2)
        o = 1 - s
        xsT = x[s][b].T.copy()          # [192, L]
        xoT = x[o][b].T.copy()
        A = -np.exp(Alog[s])            # [DM, N]
        assert np.allclose(A, A[0:1, :], atol=0), "A must be d-independent"
        Arow = A[0]                     # [N]
        WdA0 = np.zeros((16, 128, P), np.float32)
        for j in range(16):
            WdA0[j, 8 * j + dmap, np.arange(P)] = Arow[nmap]
        WdA1 = np.zeros((8, 64, P), np.float32)
        for j in range(8):
            WdA1[j, 8 * j + dmap, np.arange(P)] = Arow[nmap]
        S80 = np.zeros((16, P, 128), np.float32)
        for j in range(16):
            S80[j, np.arange(P), 8 * j + dmap] = 1.0
        S81 = np.zeros((8, P, 64), np.float32)
        for j in range(8):
            S81[j, np.arange(P), 8 * j + dmap] = 1.0
        Pm = emb[o] @ tok[o]            # [T, N]
        PmRep = np.ascontiguousarray(Pm[:, nmap])                 # [T, P]
        CrepT = np.ascontiguousarray(xproj[o][R + N:R + 2 * N][nmap].T)
        BrepT = np.ascontiguousarray(xproj[s][R:R + N][nmap].T)
        Mdt = (dtw[s] @ xproj[s][:R]).T.copy()                    # [DM, DM]

        cb_consts = {
            "w1T0": rw1[o].T[:128], "w1T1": rw1[o].T[128:], "w2T": rw2[o].T,
            "PmRep": PmRep, "xpC0": CrepT[:128], "xpC1": CrepT[128:],
            "xpB0": BrepT[:128], "xpB1": BrepT[128:],
            "Mdt0": Mdt[:128], "Mdt1": Mdt[128:],
        }
        cpb_arr = np.zeros((128, CBTOT), np.float32)
        for nm, (off, r, ccols) in CBMAP.items():
            a = np.asarray(cb_consts[nm], np.float32)
            assert a.shape == (r, ccols), (nm, a.shape)
            cpb_arr[:r, off:off + ccols] = a

        cr_consts = {
            "WdA0": np.transpose(WdA0, (1, 0, 2)).reshape(128, 16 * P),
            "WdA1": np.transpose(WdA1, (1, 0, 2)).reshape(64, 8 * P),
            "S80": np.transpose(S80, (1, 0, 2)).reshape(P, 16 * 128),
            "S81": np.transpose(S81, (1, 0, 2)).reshape(P, 8 * 64),
            "onc0": np.ones((128, 1), np.float32),
            "onc1": np.ones((64, 1), np.float32),
        }
        cpr_arr = np.zeros((128, CRTOT), np.float32)
        for nm, (off, r, ccols) in CRMAP.items():
            a = np.asarray(cr_consts[nm], np.float32)
            assert a.shape == (r, ccols), (nm, a.shape)
            cpr_arr[:r, off:off + ccols] = a

        cf_consts = {
            "identF": np.eye(128, dtype=np.float32),
            "b1c": rb1[o][:, None], "dtb0": dtb[s][:128, None],
            "dtb1": dtb[s][128:, None],
            "invA": (1.0 / Arow[nmap])[:, None],
            "Dc0": Dsk[s][:128, None], "Dc1": Dsk[s][128:, None],
            "lnb0": lnb[s][:128, None], "lnb1": lnb[s][128:, None],
            "lngr0": lng[s][None, :128], "lngr1": lng[s][None, 128:],
        }
        cpf_arr = np.zeros((128, CFTOT), np.float32)
        for nm, (off, r, ccols) in CFMAP.items():
            a = np.asarray(cf_consts[nm], np.float32)
            assert a.shape == (r, ccols), (nm, a.shape)
            cpf_arr[:r, off:off + ccols] = a

        gqa = (-np.log(-np.log(u[o][b])) + rb2[o][None, :]).astype(np.float32)
        m = {
            "xsT0": bf16(xsT[:128]), "xsT1": bf16(xsT[128:]),
            "xoT0": bf16(xoT[:128]), "xoT1": bf16(xoT[128:]),
            "gq": gqa.reshape(L // 128, 128, T).copy(),
            "cpb": np.ascontiguousarray(cpb_arr.astype(ml_dtypes.bfloat16)),
            "cpr": cpr_arr,
            "cpf": cpf_arr,
        }
        in_maps.append(m)
    return in_maps


def run(inputs, trace=False):
    nc = _get_prog()
    in_maps = _make_in_maps(inputs)
    res = run_bass_kernel_spmd(nc, in_maps, list(range(8)), trace=trace)
    out = np.zeros((2, B, DM, L), np.float32)
    for c in range(8):
        b, s = divmod(c, 2)
        out[s, b, :128] = res.results[c]["yo0"]
        out[s, b, 128:] = res.results[c]["yo1"]
    return out, res


def kernel(**inputs):
    out, _ = run(inputs, trace=False)
    return out


# revision 9
# speedup vs baseline: 1.5749x; 1.1189x over previous
"""Cross-modal selective-scan (ASSM) kernel for 8 TRN2 NeuronCores.

Sharding: one core per (batch, stream) pair: core = b*2 + s, s=0 rgb / s=1 e.
Each core computes the full forward for its stream (routing+gumbel of the
OTHER stream feeds C — cross-modal), the L=4096 selective scan over
(D=192, N=16) states, and the output layernorm. Outputs are gathered on host.

v3 highlights:
 - matmuls in bf16 (x-projections / routing / yacc) and fp32r (scan args),
   never plain fp32 on big streams -> ~4x PE throughput per column.
 - the state-injection/readout chain (dBu, h, ym, B, C) runs in bf16: DVE
   2x mode for the elementwise ops, half the SBUF traffic. The compounding
   decay dA stays fp32 (bf16 decay quantization accumulates over the scan).
 - gumbel noise -ln(-ln u) (+ b2 routing bias) precomputed on host.
 - the next chunk's load/routing/projection preamble is emitted in the
   middle of the current chunk's scan phase (software pipelining).
 - GpSimd handles SBUF-only side ops (w-build, softplus add, one-hot eq,
   y^2); it cannot access PSUM, so PSUM readers stay on Vector/Scalar.
"""

import numpy as np
import ml_dtypes

import concourse.bass as bass
import concourse.bacc as bacc
import concourse.mybir as mybir
import concourse.tile as tile
from concourse.bass_utils import run_bass_kernel_spmd

FP = mybir.dt.float32
FPR = mybir.dt.float32r
BF = mybir.dt.bfloat16
OP = mybir.AluOpType
AF = mybir.ActivationFunctionType

B, L, DM, N, R, T, H3 = 4, 4096, 192, 16, 12, 64, 64
P = 128
LC = 1024               # chunk along L
NCH = L // LC           # 4
SUB = 512
LEPS = 1e-5
NT = LC // 128          # 8 token tiles per chunk

# ---- packed-constant layouts: name -> (col offset, rows, cols) ----
def _pack(shapes):
    m, off = {}, 0
    for nm, r, c in shapes:
        m[nm] = (off, r, c)
        off += c
    return m, off

_CB_SHAPES = [
    ("w1T0", 128, 64), ("w1T1", 64, 64), ("w2T", 64, 64), ("PmRep", 64, 128),
    ("xpC0", 128, 128), ("xpC1", 64, 128), ("xpB0", 128, 128),
    ("xpB1", 64, 128), ("Mdt0", 128, 192), ("Mdt1", 64, 192),
    ("S80", 128, 16 * 128), ("S81", 128, 8 * 64),
]
CBMAP, CBTOT = _pack(_CB_SHAPES)

_CR_SHAPES = [
    ("WdA0", 128, 16 * 128), ("WdA1", 64, 8 * 128), ("onc0", 128, 1),
    ("onc1", 64, 1),
]
CRMAP, CRTOT = _pack(_CR_SHAPES)

_CF_SHAPES = [
    ("identF", 128, 128), ("b1c", 64, 1), ("dtb0", 128, 1), ("dtb1", 64, 1),
    ("invA", 128, 1), ("Dc0", 128, 1), ("Dc1", 64, 1), ("lnb0", 128, 1),
    ("lnb1", 64, 1), ("lngr0", 1, 128), ("lngr1", 1, 64),
]
CFMAP, CFTOT = _pack(_CF_SHAPES)


def build_program():
    nc = bacc.Bacc("TRN2", target_bir_lowering=False, debug=False)

    xsT0 = nc.declare_dram_parameter("xsT0", [128, L], BF, isOutput=False)
    xsT1 = nc.declare_dram_parameter("xsT1", [64, L], BF, isOutput=False)
    xoT0 = nc.declare_dram_parameter("xoT0", [128, L], BF, isOutput=False)
    xoT1 = nc.declare_dram_parameter("xoT1", [64, L], BF, isOutput=False)
    gq = nc.declare_dram_parameter("gq", [L // 128, 128, T], FP, isOutput=False)
    cpb = nc.declare_dram_parameter("cpb", [128, CBTOT], BF, isOutput=False)
    cpr = nc.declare_dram_parameter("cpr", [128, CRTOT], FPR, isOutput=False)
    cpf = nc.declare_dram_parameter("cpf", [128, CFTOT], FP, isOutput=False)
    yo0 = nc.declare_dram_parameter("yo0", [128, L], FP, isOutput=True)
    yo1 = nc.declare_dram_parameter("yo1", [64, L], FP, isOutput=True)

    with tile.TileContext(nc) as tc:
        with (
            tc.tile_pool(name="const", bufs=1) as cp,
            tc.tile_pool(name="xin", bufs=2) as xp,
            tc.tile_pool(name="dwp", bufs=2) as dwp,
            tc.tile_pool(name="proj", bufs=2) as pj,
            tc.tile_pool(name="route", bufs=2) as rt,
            tc.tile_pool(name="spool", bufs=2) as sp_,
            tc.tile_pool(name="blk", bufs=3) as bk,
            tc.tile_pool(name="hpool", bufs=3) as hp,
            tc.tile_pool(name="ypool", bufs=1) as yp,
            tc.tile_pool(name="rows", bufs=1) as rw,
            tc.tile_pool(name="persist", bufs=1) as pe_,
            tc.tile_pool(name="ps_mm", bufs=2, space="PSUM") as ps_mm,
            tc.tile_pool(name="ps_y", bufs=1, space="PSUM") as ps_y,
            tc.tile_pool(name="ps_s", bufs=2, space="PSUM") as ps_s,
        ):
            cbk = cp.tile([128, CBTOT], BF, tag="cpb")
            nc.sync.dma_start(cbk[:], cpb[:])
            crk = cp.tile([128, CRTOT], FPR, tag="cpr")
            nc.sync.dma_start(crk[:], cpr[:])
            cfk = cp.tile([128, CFTOT], FP, tag="cpf")
            nc.sync.dma_start(cfk[:], cpf[:])

            def cb(name):
                off, r, c = CBMAP[name]
                return cbk[0:r, off:off + c]

            def cr(name):
                off, r, c = CRMAP[name]
                return crk[0:r, off:off + c]

            def cf(name):
                off, r, c = CFMAP[name]
                return cfk[0:r, off:off + c]

            def mm512(out, lhsT, rhs, start, stop):
                # ISA caps the moving operand at 512 elements per matmul
                n = rhs.shape[-1]
                for q in range(0, n, 512):
                    e = min(q + 512, n)
                    nc.tensor.matmul(out[:, q:e], lhsT, rhs[:, q:e],
                                     start=start, stop=stop)

            hlast = pe_.tile([P, 24], FP)
            epsc = pe_.tile([128, 1], FP)
            nc.vector.memset(epsc[:], LEPS)

            def emit_preamble(kc):
                """Loads + routing + projections + w-build for chunk kc.
                Returns the chunk context dict."""
                ls = kc * LC
                c0 = ls // 128
                C = {}
                xs0 = C["xs0"] = xp.tile([128, LC], BF, tag="xs0", name=f"xs0_{kc}")
                xs1 = C["xs1"] = xp.tile([64, LC], BF, tag="xs1", name=f"xs1_{kc}")
                xo0 = xp.tile([128, LC], BF, tag="xo0", name=f"xo0_{kc}")
                xo1 = xp.tile([64, LC], BF, tag="xo1", name=f"xo1_{kc}")
                gt = xp.tile([128, NT * T], FP, tag="gt", name=f"gt_{kc}")
                nc.sync.dma_start(xs0[:], xsT0[:, ls:ls + LC])
                nc.sync.dma_start(xs1[:], xsT1[:, ls:ls + LC])
                nc.sync.dma_start(xo0[:], xoT0[:, ls:ls + LC])
                nc.sync.dma_start(xo1[:], xoT1[:, ls:ls + LC])
                nc.sync.dma_start(
                    gt[:].rearrange("p (c t) -> p c t", c=NT),
                    gq[c0:c0 + NT].rearrange("c p t -> p c t"))

                # routing of the other stream -> one-hot OT
                zt = ps_mm.tile([128, LC], FP, tag="mm", name=f"zt{kc}")
                mm512(zt[0:H3, :], cb("w1T0"), xo0[:], True, False)
                mm512(zt[0:H3, :], cb("w1T1"), xo1[:], False, True)
                hg = rt.tile([H3, LC], BF, tag="hg", name=f"hg_{kc}")
                nc.scalar.activation(hg[:], zt[0:H3, :], AF.Gelu,
                                     bias=cf("b1c"))
                z2 = ps_s.tile([128, NT * T], FP, tag="s", name=f"z2{kc}")
                for i in range(NT):
                    nc.tensor.matmul(z2[:, i * T:(i + 1) * T],
                                     hg[:, i * 128:(i + 1) * 128], cb("w2T"),
                                     start=True, stop=True)
                zg = rt.tile([128, NT * T], FP, tag="zg", name=f"zg_{kc}")
                nc.vector.tensor_tensor(zg[:], z2[:], gt[:], OP.add)
                oh = rt.tile([128, NT * T], FP, tag="oh", name=f"oh_{kc}")
                for i in range(NT):
                    sl = slice(i * T, (i + 1) * T)
                    m8 = rt.tile([128, 8], FP, tag="m8", bufs=3, name=f"m8_{kc}_{i}")
                    nc.vector.max(m8[:], zg[:, sl])
                    nc.gpsimd.tensor_scalar(oh[:, sl], zg[:, sl], m8[:, 0:1],
                                            None, OP.is_equal)
                tp = ps_mm.tile([128, LC], FP, tag="mm", name=f"tp{kc}")
                for i in range(NT):
                    nc.tensor.transpose(tp[0:T, i * 128:(i + 1) * 128],
                                        oh[:, i * T:(i + 1) * T], cf("identF"))
                OT = rt.tile([T, LC], BF, tag="OT", name=f"OT_{kc}")
                nc.scalar.copy(OT[:], tp[0:T, :])

                # dt -> softplus -> dl ; w = dl*x   (packed [dl|w] per sub)
                dw0 = C["dw0"] = dwp.tile([128, 2 * LC], FPR, tag="dw0", name=f"dw0_{kc}")
                dw1 = C["dw1"] = dwp.tile([64, 2 * LC], FPR, tag="dw1", name=f"dw1_{kc}")
                dv0 = dw0[:].rearrange("p (s h k) -> p s h k", s=2, h=2)
                dv1 = dw1[:].rearrange("p (s h k) -> p s h k", s=2, h=2)

                dtp0 = ps_mm.tile([128, LC], FP, tag="mm", name=f"dt0{kc}")
                mm512(dtp0[:], cb("Mdt0")[:, 0:128], xs0[:], True, False)
                mm512(dtp0[:], cb("Mdt1")[:, 0:128], xs1[:], False, True)
                dtp1 = ps_mm.tile([64, LC], FP, tag="mm", name=f"dt1{kc}")
                mm512(dtp1[:], cb("Mdt0")[:, 128:DM], xs0[:], True, False)
                mm512(dtp1[:], cb("Mdt1")[:, 128:DM], xs1[:], False, True)
                # softplus(x) = relu(x) + ln(1 + exp(-|x|))
                sp0 = sp_.tile([128, LC], FP, tag="sp0", name=f"sp0_{kc}")
                sp1 = sp_.tile([64, LC], FP, tag="sp1", name=f"sp1_{kc}")
                dtv0 = dtp0[:].rearrange("p (s k) -> p s k", s=2)
                dtv1 = dtp1[:].rearrange("p (s k) -> p s k", s=2)
                nc.scalar.activation(sp0[:], dtp0[:], AF.Abs, bias=cf("dtb0"))
                nc.scalar.activation(sp1[:], dtp1[:], AF.Abs, bias=cf("dtb1"))
                nc.scalar.activation(sp0[:], sp0[:], AF.Exp, scale=-1.0)
                nc.scalar.activation(sp1[:], sp1[:], AF.Exp, scale=-1.0)
                nc.scalar.activation(sp0[:], sp0[:], AF.Ln, bias=1.0)
                nc.scalar.activation(sp1[:], sp1[:], AF.Ln, bias=1.0)
                nc.scalar.activation(dv0[:, :, 0, :], dtv0[:], AF.Relu,
                                     bias=cf("dtb0"))
                nc.scalar.activation(dv1[:, :, 0, :], dtv1[:], AF.Relu,
                                     bias=cf("dtb1"))
                sv0 = sp0[:].rearrange("p (s k) -> p s k", s=2)
                sv1 = sp1[:].rearrange("p (s k) -> p s k", s=2)
                nc.gpsimd.tensor_tensor(dv0[:, :, 0, :],
                                        dv0[:, :, 0, :].bitcast(FP),
                                        sv0[:], OP.add)
                nc.gpsimd.tensor_tensor(dv1[:, :, 0, :],
                                        dv1[:, :, 0, :].bitcast(FP),
                                        sv1[:], OP.add)
                xv0 = xs0[:].rearrange("p (s k) -> p s k", s=2)
                xv1 = xs1[:].rearrange("p (s k) -> p s k", s=2)
                nc.gpsimd.tensor_tensor(dv0[:, :, 1, :],
                                        dv0[:, :, 0, :].bitcast(FP),
                                        xv0[:], OP.mult)
                nc.gpsimd.tensor_tensor(dv1[:, :, 1, :],
                                        dv1[:, :, 0, :].bitcast(FP),
                                        xv1[:], OP.mult)

                bp = ps_mm.tile([128, LC], FP, tag="mm", name=f"bp{kc}")
                mm512(bp[:], cb("xpB0"), xs0[:], True, False)
                mm512(bp[:], cb("xpB1"), xs1[:], False, True)
                Brep = C["Brep"] = pj.tile([128, LC], BF, tag="Brep", name=f"Brep_{kc}")
                nc.scalar.copy(Brep[:], bp[:])
                cpp = ps_mm.tile([128, LC], FP, tag="mm", name=f"cp{kc}")
                mm512(cpp[:], cb("xpC0"), xo0[:], True, False)
                mm512(cpp[:], cb("xpC1"), xo1[:], False, False)
                mm512(cpp[:], cb("PmRep"), OT[:], False, True)
                Crep = C["Crep"] = pj.tile([128, LC], BF, tag="Crep", name=f"Crep_{kc}")
                nc.scalar.copy(Crep[:], cpp[:])
                C["kc"] = kc
                C["hl_pend"] = []
                return C

            def emit_block(C, j):
                kc = C["kc"]
                if j < 16:
                    dwt = C["dw0"]
                    wa = cr("WdA0")[:, j * P:(j + 1) * P]
                    sc = cb("S80")[:, j * 128:(j + 1) * 128]
                    yac, rows_ = C["yac0"], P
                    first, last = j == 0, j == 15
                else:
                    dwt = C["dw1"]
                    wa = cr("WdA1")[:, (j - 16) * P:(j - 15) * P]
                    sc = cb("S81")[:, (j - 16) * 64:(j - 15) * 64]
                    yac, rows_ = C["yac1"], 64
                    first, last = j == 16, j == 23
                # deferred hlast copies (Act) — 2 blocks late so Act never
                # stalls waiting for the scan of the current block
                while C["hl_pend"] and C["hl_pend"][0][0] <= j - 2:
                    _, jj, hh = C["hl_pend"].pop(0)
                    nc.scalar.copy(hlast[:, jj:jj + 1], hh[:, LC - 1:LC])
                dA = bk.tile([P, LC], FP, tag="dA", name=f"dA_{kc}_{j}")
                dBu = bk.tile([P, LC], BF, tag="dBu", name=f"dBu_{kc}_{j}")
                for sb in range(2):
                    mmp = ps_mm.tile([128, LC], FP, tag="mm",
                                     name=f"mm{kc}_{j}_{sb}")
                    mm512(mmp[:], wa, dwt[:, sb * LC:(sb + 1) * LC],
                          True, True)
                    ssl = slice(sb * SUB, (sb + 1) * SUB)
                    nc.scalar.activation(dA[:, ssl], mmp[:, 0:SUB], AF.Exp)
                    wcp = bk.tile([P, SUB], BF, tag="wcp", name=f"wcp_{kc}_{j}_{sb}")
                    nc.scalar.activation(wcp[:], mmp[:, SUB:LC], AF.Copy,
                                         scale=cf("invA"))
                    nc.vector.tensor_tensor(dBu[:, ssl], wcp[:],
                                            C["Brep"][:, ssl], OP.mult)
                h = hp.tile([P, LC], BF, tag="h", name=f"h_{kc}_{j}")
                init = 0.0 if kc == 0 else hlast[:, j:j + 1]
                nc.vector.tensor_tensor_scan(h[:], dA[:], dBu[:], init,
                                             OP.mult, OP.add)
                if kc < NCH - 1:
                    C["hl_pend"].append((j, j, h))
                ym = hp.tile([P, LC], BF, tag="ym", name=f"ym_{kc}_{j}")
                nc.vector.tensor_tensor(ym[:], h[:], C["Crep"][:], OP.mult)
                mm512(yac[0:rows_, :], sc, ym[:], first, last)
                if j == 15:
                    yD0 = C["yD0"] = yp.tile([128, LC], FPR, tag="yD0", name=f"yD0_{kc}")
                    nc.vector.scalar_tensor_tensor(
                        yD0[:], C["xs0"][:], cf("Dc0"), C["yac0"][:],
                        OP.mult, OP.add)
                    C["yac1"] = ps_y.tile([64, LC], FP, tag="y",
                                          name=f"y1_{kc}")

            def emit_ln(C):
                kc = C["kc"]
                ls = kc * LC
                while C["hl_pend"]:
                    _, jj, hh = C["hl_pend"].pop(0)
                    nc.scalar.copy(hlast[:, jj:jj + 1], hh[:, LC - 1:LC])
                yD0 = C["yD0"]
                yD1 = yp.tile([64, LC], FPR, tag="yD1", name=f"yD1_{kc}")
                nc.vector.scalar_tensor_tensor(
                    yD1[:], C["xs1"][:], cf("Dc1"), C["yac1"][:],
                    OP.mult, OP.add)
                ysq0 = yp.tile([128, LC], FPR, tag="ysq0", name=f"ysq0_{kc}")
                nc.gpsimd.tensor_tensor(ysq0[:], yD0[:].bitcast(FP),
                                        yD0[:].bitcast(FP), OP.mult)
                ysq1 = yp.tile([64, LC], FPR, tag="ysq1", name=f"ysq1_{kc}")
                nc.gpsimd.tensor_tensor(ysq1[:], yD1[:].bitcast(FP),
                                        yD1[:].bitcast(FP), OP.mult)

                s1p = ps_mm.tile([128, LC], FP, tag="mm", name=f"s1{kc}")
                mm512(s1p[0:1, :], cr("onc0"), yD0[:], True, False)
                mm512(s1p[0:1, :], cr("onc1"), yD1[:], False, True)
                s2p = ps_mm.tile([128, LC], FP, tag="mm", name=f"s2{kc}")
                mm512(s2p[0:1, :], cr("onc0"), ysq0[:], True, False)
                mm512(s2p[0:1, :], cr("onc1"), ysq1[:], False, True)
                s1row = rw.tile([1, LC], FP, tag="s1row", name=f"s1row_{kc}")
                s2row = rw.tile([1, LC], FP, tag="s2row", name=f"s2row_{kc}")
                nc.scalar.copy(s1row[:], s1p[0:1, :])
                nc.scalar.copy(s2row[:], s2p[0:1, :])
                nw = LC // 128
                sm = rw.tile([128, 2 * nw], FP, tag="sm", name=f"sm_{kc}")
                nc.sync.dma_start(sm[:, 0:nw], s1row[:])
                nc.sync.dma_start(sm[:, nw:2 * nw], s2row[:])
                mu = rw.tile([128, nw], FP, tag="mu", name=f"mu_{kc}")
                nc.vector.tensor_scalar(mu[:], sm[:, 0:nw], 1.0 / DM, None,
                                        OP.mult)
                musq = rw.tile([128, nw], FP, tag="musq", name=f"musq_{kc}")
                nc.scalar.activation(musq[:], mu[:], AF.Square)
                var = rw.tile([128, nw], FP, tag="var", name=f"var_{kc}")
                nc.vector.scalar_tensor_tensor(
                    var[:], sm[:, nw:2 * nw], 1.0 / DM, musq[:],
                    OP.mult, OP.subtract)
                sd = rw.tile([128, nw], FP, tag="sd", name=f"sd_{kc}")
                nc.scalar.activation(sd[:], var[:], AF.Sqrt, bias=epsc[:])
                inv = rw.tile([128, nw], FP, tag="inv", name=f"inv_{kc}")
                nc.vector.reciprocal(inv[:], sd[:])
                mui = rw.tile([128, nw], FP, tag="mui", name=f"mui_{kc}")
                nc.vector.tensor_tensor(mui[:], mu[:], inv[:], OP.mult)
                irow = rw.tile([1, LC], FP, tag="irow", name=f"irow_{kc}")
                nc.sync.dma_start(irow[:], inv[:])
                mirow = rw.tile([1, LC], FP, tag="mirow", name=f"mirow_{kc}")
                nc.sync.dma_start(mirow[:], mui[:])

                # broadcast g*inv and g*mu*inv via k=1 fp32 matmuls
                ib0 = ps_mm.tile([128, LC], FP, tag="mm", name=f"ib0{kc}")
                mi0 = ps_mm.tile([128, LC], FP, tag="mm", name=f"mi0{kc}")
                mm512(ib0[:], cf("lngr0"), irow[:], True, True)
                mm512(mi0[:], cf("lngr0"), mirow[:], True, True)
                yo0t = yp.tile([128, LC], FP, tag="yo0t", name=f"yo0t_{kc}")
                nc.vector.tensor_tensor(yo0t[:], yD0[:].bitcast(FP), ib0[:],
                                        OP.mult)
                nc.vector.scalar_tensor_tensor(
                    yo0t[:], yo0t[:], cf("lnb0"), mi0[:], OP.add, OP.subtract)
                nc.sync.dma_start(yo0[:, ls:ls + LC], yo0t[:])

                ib1 = ps_mm.tile([64, LC], FP, tag="mm", name=f"ib1{kc}")
                mi1 = ps_mm.tile([64, LC], FP, tag="mm", name=f"mi1{kc}")
                mm512(ib1[:], cf("lngr1"), irow[:], True, True)
                mm512(mi1[:], cf("lngr1"), mirow[:], True, True)
                yo1t = yp.tile([64, LC], FP, tag="yo1t", name=f"yo1t_{kc}")
                nc.vector.tensor_tensor(yo1t[:], yD1[:].bitcast(FP), ib1[:],
                                        OP.mult)
                nc.vector.scalar_tensor_tensor(
                    yo1t[:], yo1t[:], cf("lnb1"), mi1[:], OP.add, OP.subtract)
                nc.sync.dma_start(yo1[:, ls:ls + LC], yo1t[:])

            # ---- software-pipelined chunk loop ----
            Ccur = emit_preamble(0)
            Ccur["yac0"] = ps_y.tile([128, LC], FP, tag="y", name="y0_0")
            for kc in range(NCH):
                for j in range(8):
                    emit_block(Ccur, j)
                Cnext = emit_preamble(kc + 1) if kc + 1 < NCH else None
                for j in range(8, 24):
                    emit_block(Ccur, j)
                if Cnext is not None:
                    Cnext["yac0"] = ps_y.tile([128, LC], FP, tag="y",
                                              name=f"y0_{kc + 1}")
                emit_ln(Ccur)
                Ccur = Cnext

    nc.compile()
    return nc


_PROG = None


def _get_prog():
    global _PROG
    if _PROG is None:
        _PROG = build_program()
    return _PROG


def _make_in_maps(inputs):
    f32 = lambda a: np.ascontiguousarray(np.asarray(a, dtype=np.float32))
    bf16 = lambda a: np.ascontiguousarray(
        np.asarray(np.asarray(a, dtype=np.float32), dtype=ml_dtypes.bfloat16))
    x = {0: f32(inputs["x_rgb"]), 1: f32(inputs["x_e"])}
    u = {0: f32(inputs["u_rgb"]), 1: f32(inputs["u_e"])}
    rw1 = {0: f32(inputs["route_rgb_w1"]), 1: f32(inputs["route_e_w1"])}
    rb1 = {0: f32(inputs["route_rgb_b1"]), 1: f32(inputs["route_e_b1"])}
    rw2 = {0: f32(inputs["route_rgb_w2"]), 1: f32(inputs["route_e_w2"])}
    rb2 = {0: f32(inputs["route_rgb_b2"]), 1: f32(inputs["route_e_b2"])}
    emb = {0: f32(inputs["emb_rgb"]), 1: f32(inputs["emb_e"])}
    tok = {0: f32(inputs["token_rgb_w"]), 1: f32(inputs["token_e_w"])}
    xproj = {0: f32(inputs["xproj_rgb"]), 1: f32(inputs["xproj_e"])}
    dtw = {0: f32(inputs["dtw_rgb"]), 1: f32(inputs["dtw_e"])}
    dtb = {0: f32(inputs["dtb_rgb"]), 1: f32(inputs["dtb_e"])}
    Alog = {0: f32(inputs["Alog_rgb"]), 1: f32(inputs["Alog_e"])}
    Dsk = {0: f32(inputs["D_rgb"]), 1: f32(inputs["D_e"])}
    lng = {0: f32(inputs["ln1_g"]), 1: f32(inputs["ln2_g"])}
    lnb = {0: f32(inputs["ln1_b"]), 1: f32(inputs["ln2_b"])}

    nmap = np.arange(P) % 16   # p -> n
    dmap = np.arange(P) // 16  # p -> d8

    in_maps = []
    for c in range(8):
        b, s = divmod(c, 2)
        o = 1 - s
        xsT = x[s][b].T.copy()          # [192, L]
        xoT = x[o][b].T.copy()
        A = -np.exp(Alog[s])            # [DM, N]
        assert np.allclose(A, A[0:1, :], atol=0), "A must be d-independent"
        Arow = A[0]                     # [N]
        WdA0 = np.zeros((16, 128, P), np.float32)
        for j in range(16):
            WdA0[j, 8 * j + dmap, np.arange(P)] = Arow[nmap]
        WdA1 = np.zeros((8, 64, P), np.float32)
        for j in range(8):
            WdA1[j, 8 * j + dmap, np.arange(P)] = Arow[nmap]
        S80 = np.zeros((16, P, 128), np.float32)
        for j in range(16):
            S80[j, np.arange(P), 8 * j + dmap] = 1.0
        S81 = np.zeros((8, P, 64), np.float32)
        for j in range(8):
            S81[j, np.arange(P), 8 * j + dmap] = 1.0
        Pm = emb[o] @ tok[o]            # [T, N]
        PmRep = np.ascontiguousarray(Pm[:, nmap])                 # [T, P]
        CrepT = np.ascontiguousarray(xproj[o][R + N:R + 2 * N][nmap].T)
        BrepT = np.ascontiguousarray(xproj[s][R:R + N][nmap].T)
        Mdt = (dtw[s] @ xproj[s][:R]).T.copy()                    # [DM, DM]

        cb_consts = {
            "w1T0": rw1[o].T[:128], "w1T1": rw1[o].T[128:], "w2T": rw2[o].T,
            "PmRep": PmRep, "xpC0": CrepT[:128], "xpC1": CrepT[128:],
            "xpB0": BrepT[:128], "xpB1": BrepT[128:],
            "Mdt0": Mdt[:128], "Mdt1": Mdt[128:],
            "S80": np.transpose(S80, (1, 0, 2)).reshape(P, 16 * 128),
            "S81": np.transpose(S81, (1, 0, 2)).reshape(P, 8 * 64),
        }
        cpb_arr = np.zeros((128, CBTOT), np.float32)
        for nm, (off, r, ccols) in CBMAP.items():
            a = np.asarray(cb_consts[nm], np.float32)
            assert a.shape == (r, ccols), (nm, a.shape)
            cpb_arr[:r, off:off + ccols] = a

        cr_consts = {
            "WdA0": np.transpose(WdA0, (1, 0, 2)).reshape(128, 16 * P),
            "WdA1": np.transpose(WdA1, (1, 0, 2)).reshape(64, 8 * P),
            "onc0": np.ones((128, 1), np.float32),
            "onc1": np.ones((64, 1), np.float32),
        }
        cpr_arr = np.zeros((128, CRTOT), np.float32)
        for nm, (off, r, ccols) in CRMAP.items():
            a = np.asarray(cr_consts[nm], np.float32)
            assert a.shape == (r, ccols), (nm, a.shape)
            cpr_arr[:r, off:off + ccols] = a

        cf_consts = {
            "identF": np.eye(128, dtype=np.float32),
            "b1c": rb1[o][:, None], "dtb0": dtb[s][:128, None],
            "dtb1": dtb[s][128:, None],
            "invA": (1.0 / Arow[nmap])[:, None],
            "Dc0": Dsk[s][:128, None], "Dc1": Dsk[s][128:, None],
            "lnb0": lnb[s][:128, None], "lnb1": lnb[s][128:, None],
            "lngr0": lng[s][None, :128], "lngr1": lng[s][None, 128:],
        }
        cpf_arr = np.zeros((128, CFTOT), np.float32)
        for nm, (off, r, ccols) in CFMAP.items():
            a = np.asarray(cf_consts[nm], np.float32)
            assert a.shape == (r, ccols), (nm, a.shape)
            cpf_arr[:r, off:off + ccols] = a

        gqa = (-np.log(-np.log(u[o][b])) + rb2[o][None, :]).astype(np.float32)
        m = {
            "xsT0": bf16(xsT[:128]), "xsT1": bf16(xsT[128:]),
            "xoT0": bf16(xoT[:128]), "xoT1": bf16(xoT[128:]),
            "gq": gqa.reshape(L // 128, 128, T).copy(),
            "cpb": np.ascontiguousarray(cpb_arr.astype(ml_dtypes.bfloat16)),
            "cpr": cpr_arr,
            "cpf": cpf_arr,
        }
        in_maps.append(m)
    return in_maps


def run(inputs, trace=False):
    nc = _get_prog()
    in_maps = _make_in_maps(inputs)
    res = run_bass_kernel_spmd(nc, in_maps, list(range(8)), trace=trace)
    out = np.zeros((2, B, DM, L), np.float32)
    for c in range(8):
        b, s = divmod(c, 2)
        out[s, b, :128] = res.results[c]["yo0"]
        out[s, b, 128:] = res.results[c]["yo1"]
    return out, res


def kernel(**inputs):
    out, _ = run(inputs, trace=False)
    return out


# revision 11
# speedup vs baseline: 1.6961x; 1.0770x over previous
"""Cross-modal selective-scan (ASSM) kernel for 8 TRN2 NeuronCores.

Sharding: one core per (batch, stream) pair: core = b*2 + s, s=0 rgb / s=1 e.
Each core computes the full forward for its stream (routing+gumbel of the
OTHER stream feeds C — cross-modal), the L=4096 selective scan over
(D=192, N=16) states, and the output layernorm. Outputs are gathered on host.

v3 highlights:
 - matmuls in bf16 (x-projections / routing / yacc) and fp32r (scan args),
   never plain fp32 on big streams -> ~4x PE throughput per column.
 - the state-injection/readout chain (dBu, h, ym, B, C) runs in bf16: DVE
   2x mode for the elementwise ops, half the SBUF traffic. The compounding
   decay dA stays fp32 (bf16 decay quantization accumulates over the scan).
 - gumbel noise -ln(-ln u) (+ b2 routing bias) precomputed on host.
 - the next chunk's load/routing/projection preamble is emitted in the
   middle of the current chunk's scan phase (software pipelining).
 - GpSimd handles SBUF-only side ops (w-build, softplus add, one-hot eq,
   y^2); it cannot access PSUM, so PSUM readers stay on Vector/Scalar.
"""

import numpy as np
import ml_dtypes

import concourse.bass as bass
import concourse.bacc as bacc
import concourse.mybir as mybir
import concourse.tile as tile
from concourse.bass_utils import run_bass_kernel_spmd

FP = mybir.dt.float32
FPR = mybir.dt.float32r
BF = mybir.dt.bfloat16
OP = mybir.AluOpType
AF = mybir.ActivationFunctionType

B, L, DM, N, R, T, H3 = 4, 4096, 192, 16, 12, 64, 64
P = 128
LC = 1024               # chunk along L
NCH = L // LC           # 4
SUB = 512
LEPS = 1e-5
NT = LC // 128          # 8 token tiles per chunk

# ---- packed-constant layouts: name -> (col offset, rows, cols) ----
def _pack(shapes):
    m, off = {}, 0
    for nm, r, c in shapes:
        m[nm] = (off, r, c)
        off += c
    return m, off

_CB_SHAPES = [
    ("w1T0", 128, 64), ("w1T1", 64, 64), ("w2T", 64, 64), ("PmRep", 64, 128),
    ("xpC0", 128, 128), ("xpC1", 64, 128), ("xpB0", 128, 128),
    ("xpB1", 64, 128), ("Mdt0", 128, 192), ("Mdt1", 64, 192),
    ("S80", 128, 16 * 128), ("S81", 128, 8 * 64),
]
CBMAP, CBTOT = _pack(_CB_SHAPES)

_CR_SHAPES = [
    ("WdA0", 128, 16 * 128), ("WdA1", 64, 8 * 128), ("onc0", 128, 1),
    ("onc1", 64, 1),
]
CRMAP, CRTOT = _pack(_CR_SHAPES)

_CF_SHAPES = [
    ("identF", 128, 128), ("b1c", 64, 1), ("dtb0", 128, 1), ("dtb1", 64, 1),
    ("invA", 128, 1), ("Dc0", 128, 1), ("Dc1", 64, 1), ("lnb0", 128, 1),
    ("lnb1", 64, 1), ("lngr0", 1, 128), ("lngr1", 1, 64),
]
CFMAP, CFTOT = _pack(_CF_SHAPES)


def build_program():
    nc = bacc.Bacc("TRN2", target_bir_lowering=False, debug=False)

    xsT0 = nc.declare_dram_parameter("xsT0", [128, L], BF, isOutput=False)
    xsT1 = nc.declare_dram_parameter("xsT1", [64, L], BF, isOutput=False)
    xoT0 = nc.declare_dram_parameter("xoT0", [128, L], BF, isOutput=False)
    xoT1 = nc.declare_dram_parameter("xoT1", [64, L], BF, isOutput=False)
    gq = nc.declare_dram_parameter("gq", [L // 128, 128, T], FP, isOutput=False)
    cpb = nc.declare_dram_parameter("cpb", [128, CBTOT], BF, isOutput=False)
    cpr = nc.declare_dram_parameter("cpr", [128, CRTOT], FPR, isOutput=False)
    cpf = nc.declare_dram_parameter("cpf", [128, CFTOT], FP, isOutput=False)
    yo0 = nc.declare_dram_parameter("yo0", [128, L], FP, isOutput=True)
    yo1 = nc.declare_dram_parameter("yo1", [64, L], FP, isOutput=True)

    with tile.TileContext(nc) as tc:
        with (
            tc.tile_pool(name="const", bufs=1) as cp,
            tc.tile_pool(name="xin", bufs=2) as xp,
            tc.tile_pool(name="dwp", bufs=2) as dwp,
            tc.tile_pool(name="proj", bufs=2) as pj,
            tc.tile_pool(name="route", bufs=2) as rt,
            tc.tile_pool(name="spool", bufs=2) as sp_,
            tc.tile_pool(name="blk", bufs=3) as bk,
            tc.tile_pool(name="hpool", bufs=3) as hp,
            tc.tile_pool(name="ypool", bufs=1) as yp,
            tc.tile_pool(name="rows", bufs=1) as rw,
            tc.tile_pool(name="persist", bufs=1) as pe_,
            tc.tile_pool(name="ps_scan", bufs=2, space="PSUM") as ps_scan,
            tc.tile_pool(name="ps_pre", bufs=1, space="PSUM") as ps_pre,
            tc.tile_pool(name="ps_y", bufs=1, space="PSUM") as ps_y,
        ):
            cbk = cp.tile([128, CBTOT], BF, tag="cpb")
            nc.sync.dma_start(cbk[:], cpb[:])
            crk = cp.tile([128, CRTOT], FPR, tag="cpr")
            nc.sync.dma_start(crk[:], cpr[:])
            cfk = cp.tile([128, CFTOT], FP, tag="cpf")
            nc.sync.dma_start(cfk[:], cpf[:])

            def cb(name):
                off, r, c = CBMAP[name]
                return cbk[0:r, off:off + c]

            def cr(name):
                off, r, c = CRMAP[name]
                return crk[0:r, off:off + c]

            def cf(name):
                off, r, c = CFMAP[name]
                return cfk[0:r, off:off + c]

            def mm512(out, lhsT, rhs, start, stop):
                # ISA caps the moving operand at 512 elements per matmul
                n = rhs.shape[-1]
                for q in range(0, n, 512):
                    e = min(q + 512, n)
                    nc.tensor.matmul(out[:, q:e], lhsT, rhs[:, q:e],
                                     start=start, stop=stop)

            hlast = pe_.tile([P, 24], FP)
            epsc = pe_.tile([128, 1], FP)
            nc.vector.memset(epsc[:], LEPS)

            def emit_preamble(kc):
                """Loads + projections + routing + w-build for chunk kc."""
                ls = kc * LC
                c0 = ls // 128
                C = {}
                xs0 = C["xs0"] = xp.tile([128, LC], BF, tag="xs0", name=f"xs0_{kc}")
                xs1 = C["xs1"] = xp.tile([64, LC], BF, tag="xs1", name=f"xs1_{kc}")
                xo0 = xp.tile([128, LC], BF, tag="xo0", name=f"xo0_{kc}")
                xo1 = xp.tile([64, LC], BF, tag="xo1", name=f"xo1_{kc}")
                gt = xp.tile([128, NT * T], FP, tag="gt", name=f"gt_{kc}")
                nc.sync.dma_start(xs0[:], xsT0[:, ls:ls + LC])
                nc.sync.dma_start(xs1[:], xsT1[:, ls:ls + LC])
                nc.sync.dma_start(xo0[:], xoT0[:, ls:ls + LC])
                nc.sync.dma_start(xo1[:], xoT1[:, ls:ls + LC])
                nc.sync.dma_start(
                    gt[:].rearrange("p (c t) -> p c t", c=NT),
                    gq[c0:c0 + NT].rearrange("c p t -> p c t"))

                # dt -> softplus -> dl ; w = dl*x   (dw = [dl | w])
                dw0 = C["dw0"] = dwp.tile([128, 2 * LC], FPR, tag="dw0", name=f"dw0_{kc}")
                dw1 = C["dw1"] = dwp.tile([64, 2 * LC], FPR, tag="dw1", name=f"dw1_{kc}")
                dtp0 = ps_pre.tile([128, LC], FP, tag="pre", name=f"dt0{kc}")
                mm512(dtp0[:], cb("Mdt0")[:, 0:128], xs0[:], True, False)
                mm512(dtp0[:], cb("Mdt1")[:, 0:128], xs1[:], False, True)
                # softplus(x) = ln(exp(x) + 1); x = dt + dtb stays < ~3 here
                sp0 = sp_.tile([128, LC], FP, tag="sp0", name=f"sp0_{kc}")
                nc.scalar.activation(sp0[:], dtp0[:], AF.Exp, bias=cf("dtb0"))
                nc.scalar.activation(dw0[:, 0:LC], sp0[:], AF.Ln, bias=1.0)
                dtp1 = ps_pre.tile([64, LC], FP, tag="pre", name=f"dt1{kc}")
                mm512(dtp1[:], cb("Mdt0")[:, 128:DM], xs0[:], True, False)
                mm512(dtp1[:], cb("Mdt1")[:, 128:DM], xs1[:], False, True)
                sp1 = sp_.tile([64, LC], FP, tag="sp1", name=f"sp1_{kc}")
                nc.scalar.activation(sp1[:], dtp1[:], AF.Exp, bias=cf("dtb1"))
                nc.scalar.activation(dw1[:, 0:LC], sp1[:], AF.Ln, bias=1.0)
                nc.gpsimd.tensor_tensor(dw0[:, LC:2 * LC], dw0[:, 0:LC].bitcast(FP),
                                        xs0[:], OP.mult)
                nc.gpsimd.tensor_tensor(dw1[:, LC:2 * LC], dw1[:, 0:LC].bitcast(FP),
                                        xs1[:], OP.mult)

                bp = ps_pre.tile([128, LC], FP, tag="pre", name=f"bp{kc}")
                mm512(bp[:], cb("xpB0"), xs0[:], True, False)
                mm512(bp[:], cb("xpB1"), xs1[:], False, True)
                Brep = C["Brep"] = pj.tile([128, LC], BF, tag="Brep", name=f"Brep_{kc}")
                nc.scalar.copy(Brep[:], bp[:])

                # routing of the other stream -> one-hot OT
                zt = ps_pre.tile([128, LC], FP, tag="pre", name=f"zt{kc}")
                mm512(zt[0:H3, :], cb("w1T0"), xo0[:], True, False)
                mm512(zt[0:H3, :], cb("w1T1"), xo1[:], False, True)
                hg = rt.tile([H3, LC], BF, tag="hg", name=f"hg_{kc}")
                nc.scalar.activation(hg[:], zt[0:H3, :], AF.Gelu,
                                     bias=cf("b1c"))
                z2 = ps_scan.tile([128, NT * T], FP, tag="scan", name=f"z2{kc}")
                for i in range(NT):
                    nc.tensor.matmul(z2[:, i * T:(i + 1) * T],
                                     hg[:, i * 128:(i + 1) * 128], cb("w2T"),
                                     start=True, stop=True)
                zg = rt.tile([128, NT * T], FP, tag="zg", name=f"zg_{kc}")
                nc.vector.tensor_tensor(zg[:], z2[:], gt[:], OP.add)
                oh = rt.tile([128, NT * T], FP, tag="oh", name=f"oh_{kc}")
                for i in range(NT):
                    sl = slice(i * T, (i + 1) * T)
                    m8 = rt.tile([128, 8], FP, tag="m8", bufs=3, name=f"m8_{kc}_{i}")
                    nc.vector.max(m8[:], zg[:, sl])
                    nc.vector.tensor_scalar(oh[:, sl], zg[:, sl], m8[:, 0:1],
                                            None, OP.is_equal)
                tp = ps_pre.tile([128, LC], FP, tag="pre", name=f"tp{kc}")
                for i in range(NT):
                    nc.tensor.transpose(tp[0:T, i * 128:(i + 1) * 128],
                                        oh[:, i * T:(i + 1) * T], cf("identF"))
                OT = rt.tile([T, LC], BF, tag="OT", name=f"OT_{kc}")
                nc.scalar.copy(OT[:], tp[0:T, :])

                cpp = ps_pre.tile([128, LC], FP, tag="pre", name=f"cp{kc}")
                mm512(cpp[:], cb("xpC0"), xo0[:], True, False)
                mm512(cpp[:], cb("xpC1"), xo1[:], False, False)
                mm512(cpp[:], cb("PmRep"), OT[:], False, True)
                Crep = C["Crep"] = pj.tile([128, LC], BF, tag="Crep", name=f"Crep_{kc}")
                nc.scalar.copy(Crep[:], cpp[:])
                C["kc"] = kc
                C["hl_pend"] = []
                return C

            def emit_block(C, j):
                kc = C["kc"]
                if j < 16:
                    dwt = C["dw0"]
                    wa = cr("WdA0")[:, j * P:(j + 1) * P]
                    sc = cb("S80")[:, j * 128:(j + 1) * 128]
                    yac, rows_ = C["yac0"], P
                    first, last = j == 0, j == 15
                else:
                    dwt = C["dw1"]
                    wa = cr("WdA1")[:, (j - 16) * P:(j - 15) * P]
                    sc = cb("S81")[:, (j - 16) * 64:(j - 15) * 64]
                    yac, rows_ = C["yac1"], 64
                    first, last = j == 16, j == 23
                # deferred hlast copies (Act) — 2 blocks late so Act never
                # stalls waiting for the scan of the current block
                while C["hl_pend"] and C["hl_pend"][0][0] <= j - 2:
                    _, jj, hh = C["hl_pend"].pop(0)
                    nc.scalar.copy(hlast[:, jj:jj + 1], hh[:, LC - 1:LC])
                mmpA = ps_scan.tile([128, LC], FP, tag="scan",
                                    name=f"mmA{kc}_{j}")
                mm512(mmpA[:], wa, dwt[:, 0:LC], True, True)
                dA = bk.tile([P, LC], FP, tag="dA", name=f"dA_{kc}_{j}")
                nc.scalar.activation(dA[:], mmpA[:], AF.Exp)
                mmpB = ps_scan.tile([128, LC], FP, tag="scan",
                                    name=f"mmB{kc}_{j}")
                mm512(mmpB[:], wa, dwt[:, LC:2 * LC], True, True)
                wcp = bk.tile([P, LC], BF, tag="wcp", name=f"wcp_{kc}_{j}")
                nc.scalar.activation(wcp[:], mmpB[:], AF.Copy,
                                     scale=cf("invA"))
                dBu = bk.tile([P, LC], BF, tag="dBu", name=f"dBu_{kc}_{j}")
                nc.vector.tensor_tensor(dBu[:], wcp[:], C["Brep"][:], OP.mult)
                h = hp.tile([P, LC], BF, tag="h", name=f"h_{kc}_{j}")
                init = 0.0 if kc == 0 else hlast[:, j:j + 1]
                nc.vector.tensor_tensor_scan(h[:], dA[:], dBu[:], init,
                                             OP.mult, OP.add)
                if kc < NCH - 1:
                    C["hl_pend"].append((j, j, h))
                ym = hp.tile([P, LC], BF, tag="ym", name=f"ym_{kc}_{j}")
                eng = nc.gpsimd if j % 2 == 0 else nc.vector
                eng.tensor_tensor(ym[:], h[:], C["Crep"][:], OP.mult)
                mm512(yac[0:rows_, :], sc, ym[:], first, last)
                if j == 15:
                    yD0 = C["yD0"] = yp.tile([128, LC], FPR, tag="yD0", name=f"yD0_{kc}")
                    nc.vector.scalar_tensor_tensor(
                        yD0[:], C["xs0"][:], cf("Dc0"), C["yac0"][:],
                        OP.mult, OP.add)
                    C["yac1"] = ps_y.tile([64, LC], FP, tag="y",
                                          name=f"y1_{kc}")

            def emit_ln(C):
                kc = C["kc"]
                ls = kc * LC
                while C["hl_pend"]:
                    _, jj, hh = C["hl_pend"].pop(0)
                    nc.scalar.copy(hlast[:, jj:jj + 1], hh[:, LC - 1:LC])
                yD0 = C["yD0"]
                yD1 = yp.tile([64, LC], FPR, tag="yD1", name=f"yD1_{kc}")
                nc.vector.scalar_tensor_tensor(
                    yD1[:], C["xs1"][:], cf("Dc1"), C["yac1"][:],
                    OP.mult, OP.add)
                ysq0 = yp.tile([128, LC], FPR, tag="ysq0", name=f"ysq0_{kc}")
                nc.gpsimd.tensor_tensor(ysq0[:], yD0[:].bitcast(FP),
                                        yD0[:].bitcast(FP), OP.mult)
                ysq1 = yp.tile([64, LC], FPR, tag="ysq1", name=f"ysq1_{kc}")
                nc.gpsimd.tensor_tensor(ysq1[:], yD1[:].bitcast(FP),
                                        yD1[:].bitcast(FP), OP.mult)

                s1p = ps_scan.tile([128, LC], FP, tag="scan", name=f"s1{kc}")
                mm512(s1p[0:1, :], cr("onc0"), yD0[:], True, False)
                mm512(s1p[0:1, :], cr("onc1"), yD1[:], False, True)
                s2p = ps_scan.tile([128, LC], FP, tag="scan", name=f"s2{kc}")
                mm512(s2p[0:1, :], cr("onc0"), ysq0[:], True, False)
                mm512(s2p[0:1, :], cr("onc1"), ysq1[:], False, True)
                s1row = rw.tile([1, LC], FP, tag="s1row", name=f"s1row_{kc}")
                s2row = rw.tile([1, LC], FP, tag="s2row", name=f"s2row_{kc}")
                nc.scalar.copy(s1row[:], s1p[0:1, :])
                nc.scalar.copy(s2row[:], s2p[0:1, :])
                nw = LC // 128
                sm = rw.tile([128, 2 * nw], FP, tag="sm", name=f"sm_{kc}")
                nc.sync.dma_start(sm[:, 0:nw], s1row[:])
                nc.sync.dma_start(sm[:, nw:2 * nw], s2row[:])
                mu = rw.tile([128, nw], FP, tag="mu", name=f"mu_{kc}")
                nc.vector.tensor_scalar(mu[:], sm[:, 0:nw], 1.0 / DM, None,
                                        OP.mult)
                musq = rw.tile([128, nw], FP, tag="musq", name=f"musq_{kc}")
                nc.scalar.activation(musq[:], mu[:], AF.Square)
                var = rw.tile([128, nw], FP, tag="var", name=f"var_{kc}")
                nc.vector.scalar_tensor_tensor(
                    var[:], sm[:, nw:2 * nw], 1.0 / DM, musq[:],
                    OP.mult, OP.subtract)
                sd = rw.tile([128, nw], FP, tag="sd", name=f"sd_{kc}")
                nc.scalar.activation(sd[:], var[:], AF.Sqrt, bias=epsc[:])
                inv = rw.tile([128, nw], FP, tag="inv", name=f"inv_{kc}")
                nc.vector.reciprocal(inv[:], sd[:])
                mui = rw.tile([128, nw], FP, tag="mui", name=f"mui_{kc}")
                nc.vector.tensor_tensor(mui[:], mu[:], inv[:], OP.mult)
                irow = rw.tile([1, LC], FP, tag="irow", name=f"irow_{kc}")
                nc.sync.dma_start(irow[:], inv[:])
                mirow = rw.tile([1, LC], FP, tag="mirow", name=f"mirow_{kc}")
                nc.sync.dma_start(mirow[:], mui[:])

                # broadcast g*inv and g*mu*inv via k=1 fp32 matmuls
                ib0 = ps_scan.tile([128, LC], FP, tag="scan", name=f"ib0{kc}")
                mi0 = ps_scan.tile([128, LC], FP, tag="scan", name=f"mi0{kc}")
                mm512(ib0[:], cf("lngr0"), irow[:], True, True)
                mm512(mi0[:], cf("lngr0"), mirow[:], True, True)
                yo0t = yp.tile([128, LC], FP, tag="yo0t", name=f"yo0t_{kc}")
                nc.vector.tensor_tensor(yo0t[:], yD0[:].bitcast(FP), ib0[:],
                                        OP.mult)
                nc.vector.scalar_tensor_tensor(
                    yo0t[:], yo0t[:], cf("lnb0"), mi0[:], OP.add, OP.subtract)
                nc.sync.dma_start(yo0[:, ls:ls + LC], yo0t[:])

                ib1 = ps_scan.tile([64, LC], FP, tag="scan", name=f"ib1{kc}")
                mi1 = ps_scan.tile([64, LC], FP, tag="scan", name=f"mi1{kc}")
                mm512(ib1[:], cf("lngr1"), irow[:], True, True)
                mm512(mi1[:], cf("lngr1"), mirow[:], True, True)
                yo1t = yp.tile([64, LC], FP, tag="yo1t", name=f"yo1t_{kc}")
                nc.vector.tensor_tensor(yo1t[:], yD1[:].bitcast(FP), ib1[:],
                                        OP.mult)
                nc.vector.scalar_tensor_tensor(
                    yo1t[:], yo1t[:], cf("lnb1"), mi1[:], OP.add, OP.subtract)
                nc.sync.dma_start(yo1[:, ls:ls + LC], yo1t[:])

            # ---- software-pipelined chunk loop ----
            Ccur = emit_preamble(0)
            Ccur["yac0"] = ps_y.tile([128, LC], FP, tag="y", name="y0_0")
            for kc in range(NCH):
                for j in range(2):
                    emit_block(Ccur, j)
                Cnext = emit_preamble(kc + 1) if kc + 1 < NCH else None
                for j in range(2, 24):
                    emit_block(Ccur, j)
                if Cnext is not None:
                    Cnext["yac0"] = ps_y.tile([128, LC], FP, tag="y",
                                              name=f"y0_{kc + 1}")
                emit_ln(Ccur)
                Ccur = Cnext

    nc.compile()
    return nc


_PROG = None


def _get_prog():
    global _PROG
    if _PROG is None:
        _PROG = build_program()
    return _PROG


def _make_in_maps(inputs):
    f32 = lambda a: np.ascontiguousarray(np.asarray(a, dtype=np.float32))
    bf16 = lambda a: np.ascontiguousarray(
        np.asarray(np.asarray(a, dtype=np.float32), dtype=ml_dtypes.bfloat16))
    x = {0: f32(inputs["x_rgb"]), 1: f32(inputs["x_e"])}
    u = {0: f32(inputs["u_rgb"]), 1: f32(inputs["u_e"])}
    rw1 = {0: f32(inputs["route_rgb_w1"]), 1: f32(inputs["route_e_w1"])}
    rb1 = {0: f32(inputs["route_rgb_b1"]), 1: f32(inputs["route_e_b1"])}
    rw2 = {0: f32(inputs["route_rgb_w2"]), 1: f32(inputs["route_e_w2"])}
    rb2 = {0: f32(inputs["route_rgb_b2"]), 1: f32(inputs["route_e_b2"])}
    emb = {0: f32(inputs["emb_rgb"]), 1: f32(inputs["emb_e"])}
    tok = {0: f32(inputs["token_rgb_w"]), 1: f32(inputs["token_e_w"])}
    xproj = {0: f32(inputs["xproj_rgb"]), 1: f32(inputs["xproj_e"])}
    dtw = {0: f32(inputs["dtw_rgb"]), 1: f32(inputs["dtw_e"])}
    dtb = {0: f32(inputs["dtb_rgb"]), 1: f32(inputs["dtb_e"])}
    Alog = {0: f32(inputs["Alog_rgb"]), 1: f32(inputs["Alog_e"])}
    Dsk = {0: f32(inputs["D_rgb"]), 1: f32(inputs["D_e"])}
    lng = {0: f32(inputs["ln1_g"]), 1: f32(inputs["ln2_g"])}
    lnb = {0: f32(inputs["ln1_b"]), 1: f32(inputs["ln2_b"])}

    nmap = np.arange(P) % 16   # p -> n
    dmap = np.arange(P) // 16  # p -> d8

    in_maps = []
    for c in range(8):
        b, s = divmod(c, 2)
        o = 1 - s
        xsT = x[s][b].T.copy()          # [192, L]
        xoT = x[o][b].T.copy()
        A = -np.exp(Alog[s])            # [DM, N]
        assert np.allclose(A, A[0:1, :], atol=0), "A must be d-independent"
        Arow = A[0]                     # [N]
        WdA0 = np.zeros((16, 128, P), np.float32)
        for j in range(16):
            WdA0[j, 8 * j + dmap, np.arange(P)] = Arow[nmap]
        WdA1 = np.zeros((8, 64, P), np.float32)
        for j in range(8):
            WdA1[j, 8 * j + dmap, np.arange(P)] = Arow[nmap]
        S80 = np.zeros((16, P, 128), np.float32)
        for j in range(16):
            S80[j, np.arange(P), 8 * j + dmap] = 1.0
        S81 = np.zeros((8, P, 64), np.float32)
        for j in range(8):
            S81[j, np.arange(P), 8 * j + dmap] = 1.0
        Pm = emb[o] @ tok[o]            # [T, N]
        PmRep = np.ascontiguousarray(Pm[:, nmap])                 # [T, P]
        CrepT = np.ascontiguousarray(xproj[o][R + N:R + 2 * N][nmap].T)
        BrepT = np.ascontiguousarray(xproj[s][R:R + N][nmap].T)
        Mdt = (dtw[s] @ xproj[s][:R]).T.copy()                    # [DM, DM]

        cb_consts = {
            "w1T0": rw1[o].T[:128], "w1T1": rw1[o].T[128:], "w2T": rw2[o].T,
            "PmRep": PmRep, "xpC0": CrepT[:128], "xpC1": CrepT[128:],
            "xpB0": BrepT[:128], "xpB1": BrepT[128:],
            "Mdt0": Mdt[:128], "Mdt1": Mdt[128:],
            "S80": np.transpose(S80, (1, 0, 2)).reshape(P, 16 * 128),
            "S81": np.transpose(S81, (1, 0, 2)).reshape(P, 8 * 64),
        }
        cpb_arr = np.zeros((128, CBTOT), np.float32)
        for nm, (off, r, ccols) in CBMAP.items():
            a = np.asarray(cb_consts[nm], np.float32)
            assert a.shape == (r, ccols), (nm, a.shape)
            cpb_arr[:r, off:off + ccols] = a

        cr_consts = {
            "WdA0": np.transpose(WdA0, (1, 0, 2)).reshape(128, 16 * P),
            "WdA1": np.transpose(WdA1, (1, 0, 2)).reshape(64, 8 * P),
            "onc0": np.ones((128, 1), np.float32),
            "onc1": np.ones((64, 1), np.float32),
        }
        cpr_arr = np.zeros((128, CRTOT), np.float32)
        for nm, (off, r, ccols) in CRMAP.items():
            a = np.asarray(cr_consts[nm], np.float32)
            assert a.shape == (r, ccols), (nm, a.shape)
            cpr_arr[:r, off:off + ccols] = a

        cf_consts = {
            "identF": np.eye(128, dtype=np.float32),
            "b1c": rb1[o][:, None], "dtb0": dtb[s][:128, None],
            "dtb1": dtb[s][128:, None],
            "invA": (1.0 / Arow[nmap])[:, None],
            "Dc0": Dsk[s][:128, None], "Dc1": Dsk[s][128:, None],
            "lnb0": lnb[s][:128, None], "lnb1": lnb[s][128:, None],
            "lngr0": lng[s][None, :128], "lngr1": lng[s][None, 128:],
        }
        cpf_arr = np.zeros((128, CFTOT), np.float32)
        for nm, (off, r, ccols) in CFMAP.items():
            a = np.asarray(cf_consts[nm], np.float32)
            assert a.shape == (r, ccols), (nm, a.shape)
            cpf_arr[:r, off:off + ccols] = a

        gqa = (-np.log(-np.log(u[o][b])) + rb2[o][None, :]).astype(np.float32)
        m = {
            "xsT0": bf16(xsT[:128]), "xsT1": bf16(xsT[128:]),
            "xoT0": bf16(xoT[:128]), "xoT1": bf16(xoT[128:]),
            "gq": gqa.reshape(L // 128, 128, T).copy(),
            "cpb": np.ascontiguousarray(cpb_arr.astype(ml_dtypes.bfloat16)),
            "cpr": cpr_arr,
            "cpf": cpf_arr,
        }
        in_maps.append(m)
    return in_maps


def run(inputs, trace=False):
    nc = _get_prog()
    in_maps = _make_in_maps(inputs)
    res = run_bass_kernel_spmd(nc, in_maps, list(range(8)), trace=trace)
    out = np.zeros((2, B, DM, L), np.float32)
    for c in range(8):
        b, s = divmod(c, 2)
        out[s, b, :128] = res.results[c]["yo0"]
        out[s, b, 128:] = res.results[c]["yo1"]
    return out, res


def kernel(**inputs):
    out, _ = run(inputs, trace=False)
    return out


# revision 13
# speedup vs baseline: 1.9202x; 1.1321x over previous
"""Cross-modal selective-scan (ASSM) kernel for 8 TRN2 NeuronCores.

Sharding: one core per (batch, stream) pair: core = b*2 + s, s=0 rgb / s=1 e.
Each core computes the full forward for its stream (routing+gumbel of the
OTHER stream feeds C — cross-modal), the L=4096 selective scan over
(D=192, N=16) states, and the output layernorm. Outputs are gathered on host.

v3 highlights:
 - matmuls in bf16 (x-projections / routing / yacc) and fp32r (scan args),
   never plain fp32 on big streams -> ~4x PE throughput per column.
 - the state-injection/readout chain (dBu, h, ym, B, C) runs in bf16: DVE
   2x mode for the elementwise ops, half the SBUF traffic. The compounding
   decay dA stays fp32 (bf16 decay quantization accumulates over the scan).
 - gumbel noise -ln(-ln u) (+ b2 routing bias) precomputed on host.
 - the next chunk's load/routing/projection preamble is emitted in the
   middle of the current chunk's scan phase (software pipelining).
 - GpSimd handles SBUF-only side ops (w-build, softplus add, one-hot eq,
   y^2); it cannot access PSUM, so PSUM readers stay on Vector/Scalar.
"""

import numpy as np
import ml_dtypes

import concourse.bass as bass
import concourse.bacc as bacc
import concourse.mybir as mybir
import concourse.tile as tile
from concourse.bass_utils import run_bass_kernel_spmd

FP = mybir.dt.float32
FPR = mybir.dt.float32r
BF = mybir.dt.bfloat16
OP = mybir.AluOpType
AF = mybir.ActivationFunctionType

B, L, DM, N, R, T, H3 = 4, 4096, 192, 16, 12, 64, 64
P = 128
LC = 1024               # chunk along L
NCH = L // LC           # 4
SUB = 512
LEPS = 1e-5
NT = LC // 128          # 8 token tiles per chunk

# ---- packed-constant layouts: name -> (col offset, rows, cols) ----
def _pack(shapes):
    m, off = {}, 0
    for nm, r, c in shapes:
        m[nm] = (off, r, c)
        off += c
    return m, off

_CB_SHAPES = [
    ("w1T0", 128, 64), ("w1T1", 64, 64), ("w2T", 64, 64), ("PmRep", 64, 128),
    ("xpC0", 128, 128), ("xpC1", 64, 128), ("xpB0", 128, 128),
    ("xpB1", 64, 128), ("Mdt0", 128, 192), ("Mdt1", 64, 192),
    ("S80", 128, 16 * 128), ("S81", 128, 8 * 64),
]
CBMAP, CBTOT = _pack(_CB_SHAPES)

_CR_SHAPES = [
    ("WdA0", 128, 16 * 128), ("WdA1", 64, 8 * 128), ("onc0", 128, 1),
    ("onc1", 64, 1),
]
CRMAP, CRTOT = _pack(_CR_SHAPES)

_CF_SHAPES = [
    ("identF", 128, 128), ("b1c", 64, 1), ("dtb0", 128, 1), ("dtb1", 64, 1),
    ("invA", 128, 1), ("Dc0", 128, 1), ("Dc1", 64, 1), ("lnb0", 128, 1),
    ("lnb1", 64, 1), ("lngr0", 1, 128), ("lngr1", 1, 64),
]
CFMAP, CFTOT = _pack(_CF_SHAPES)


def build_program():
    nc = bacc.Bacc("TRN2", target_bir_lowering=False, debug=False)

    xsT0 = nc.declare_dram_parameter("xsT0", [128, L], BF, isOutput=False)
    xsT1 = nc.declare_dram_parameter("xsT1", [64, L], BF, isOutput=False)
    xoT0 = nc.declare_dram_parameter("xoT0", [128, L], BF, isOutput=False)
    xoT1 = nc.declare_dram_parameter("xoT1", [64, L], BF, isOutput=False)
    gq = nc.declare_dram_parameter("gq", [L // 128, 128, T], FP, isOutput=False)
    cpb = nc.declare_dram_parameter("cpb", [128, CBTOT], BF, isOutput=False)
    cpr = nc.declare_dram_parameter("cpr", [128, CRTOT], FPR, isOutput=False)
    cpf = nc.declare_dram_parameter("cpf", [128, CFTOT], FP, isOutput=False)
    yo0 = nc.declare_dram_parameter("yo0", [128, L], FP, isOutput=True)
    yo1 = nc.declare_dram_parameter("yo1", [64, L], FP, isOutput=True)

    with tile.TileContext(nc) as tc:
        with (
            tc.tile_pool(name="const", bufs=1) as cp,
            tc.tile_pool(name="xin", bufs=2) as xp,
            tc.tile_pool(name="dwp", bufs=2) as dwp,
            tc.tile_pool(name="proj", bufs=2) as pj,
            tc.tile_pool(name="route", bufs=2) as rt,
            tc.tile_pool(name="spool", bufs=2) as sp_,
            tc.tile_pool(name="blk", bufs=3) as bk,
            tc.tile_pool(name="hpool", bufs=3) as hp,
            tc.tile_pool(name="ypool", bufs=1) as yp,
            tc.tile_pool(name="rows", bufs=1) as rw,
            tc.tile_pool(name="persist", bufs=1) as pe_,
            tc.tile_pool(name="ps_scan", bufs=2, space="PSUM") as ps_scan,
            tc.tile_pool(name="ps_pre", bufs=1, space="PSUM") as ps_pre,
            tc.tile_pool(name="ps_y", bufs=1, space="PSUM") as ps_y,
        ):
            cbk = cp.tile([128, CBTOT], BF, tag="cpb")
            nc.sync.dma_start(cbk[:], cpb[:])
            crk = cp.tile([128, CRTOT], FPR, tag="cpr")
            nc.sync.dma_start(crk[:], cpr[:])
            cfk = cp.tile([128, CFTOT], FP, tag="cpf")
            nc.sync.dma_start(cfk[:], cpf[:])

            def cb(name):
                off, r, c = CBMAP[name]
                return cbk[0:r, off:off + c]

            def cr(name):
                off, r, c = CRMAP[name]
                return crk[0:r, off:off + c]

            def cf(name):
                off, r, c = CFMAP[name]
                return cfk[0:r, off:off + c]

            def mm512(out, lhsT, rhs, start, stop):
                # ISA caps the moving operand at 512 elements per matmul
                n = rhs.shape[-1]
                for q in range(0, n, 512):
                    e = min(q + 512, n)
                    nc.tensor.matmul(out[:, q:e], lhsT, rhs[:, q:e],
                                     start=start, stop=stop)

            hlast = pe_.tile([P, 24], FP)
            epsc = pe_.tile([128, 1], FP)
            nc.vector.memset(epsc[:], LEPS)

            def emit_preamble(kc):
                """Loads + projections + routing + w-build for chunk kc."""
                ls = kc * LC
                c0 = ls // 128
                C = {}
                xs0 = C["xs0"] = xp.tile([128, LC], BF, tag="xs0", name=f"xs0_{kc}")
                xs1 = C["xs1"] = xp.tile([64, LC], BF, tag="xs1", name=f"xs1_{kc}")
                xo0 = xp.tile([128, LC], BF, tag="xo0", name=f"xo0_{kc}")
                xo1 = xp.tile([64, LC], BF, tag="xo1", name=f"xo1_{kc}")
                gt = xp.tile([128, NT * T], FP, tag="gt", name=f"gt_{kc}")
                nc.sync.dma_start(xs0[:], xsT0[:, ls:ls + LC])
                nc.sync.dma_start(xs1[:], xsT1[:, ls:ls + LC])
                nc.sync.dma_start(xo0[:], xoT0[:, ls:ls + LC])
                nc.sync.dma_start(xo1[:], xoT1[:, ls:ls + LC])
                nc.sync.dma_start(
                    gt[:].rearrange("p (c t) -> p c t", c=NT),
                    gq[c0:c0 + NT].rearrange("c p t -> p c t"))

                # dt -> softplus -> dl ; w = dl*x   (dw = [dl | w])
                dw0 = C["dw0"] = dwp.tile([128, 2 * LC], FPR, tag="dw0", name=f"dw0_{kc}")
                dw1 = C["dw1"] = dwp.tile([64, 2 * LC], FPR, tag="dw1", name=f"dw1_{kc}")
                dtp0 = ps_pre.tile([128, LC], FP, tag="pre", name=f"dt0{kc}")
                mm512(dtp0[:], cb("Mdt0")[:, 0:128], xs0[:], True, False)
                mm512(dtp0[:], cb("Mdt1")[:, 0:128], xs1[:], False, True)
                # softplus(x) = ln(exp(x) + 1); x = dt + dtb stays < ~3 here
                sp0 = sp_.tile([128, LC], FP, tag="sp0", name=f"sp0_{kc}")
                nc.scalar.activation(sp0[:], dtp0[:], AF.Exp, bias=cf("dtb0"))
                nc.scalar.activation(dw0[:, 0:LC], sp0[:], AF.Ln, bias=1.0)
                dtp1 = ps_pre.tile([64, LC], FP, tag="pre", name=f"dt1{kc}")
                mm512(dtp1[:], cb("Mdt0")[:, 128:DM], xs0[:], True, False)
                mm512(dtp1[:], cb("Mdt1")[:, 128:DM], xs1[:], False, True)
                sp1 = sp_.tile([64, LC], FP, tag="sp1", name=f"sp1_{kc}")
                nc.scalar.activation(sp1[:], dtp1[:], AF.Exp, bias=cf("dtb1"))
                nc.scalar.activation(dw1[:, 0:LC], sp1[:], AF.Ln, bias=1.0)
                nc.gpsimd.tensor_tensor(dw0[:, LC:2 * LC], dw0[:, 0:LC].bitcast(FP),
                                        xs0[:], OP.mult)
                nc.gpsimd.tensor_tensor(dw1[:, LC:2 * LC], dw1[:, 0:LC].bitcast(FP),
                                        xs1[:], OP.mult)

                bp = ps_pre.tile([128, LC], FP, tag="pre", name=f"bp{kc}")
                mm512(bp[:], cb("xpB0"), xs0[:], True, False)
                mm512(bp[:], cb("xpB1"), xs1[:], False, True)
                Brep = C["Brep"] = pj.tile([128, LC], BF, tag="Brep", name=f"Brep_{kc}")
                nc.scalar.copy(Brep[:], bp[:])

                # routing of the other stream -> one-hot OT
                zt = ps_pre.tile([128, LC], FP, tag="pre", name=f"zt{kc}")
                mm512(zt[0:H3, :], cb("w1T0"), xo0[:], True, False)
                mm512(zt[0:H3, :], cb("w1T1"), xo1[:], False, True)
                hg = rt.tile([H3, LC], BF, tag="hg", name=f"hg_{kc}")
                nc.scalar.activation(hg[:], zt[0:H3, :], AF.Gelu,
                                     bias=cf("b1c"))
                z2 = ps_scan.tile([128, NT * T], FP, tag="scan", name=f"z2{kc}")
                for i in range(NT):
                    nc.tensor.matmul(z2[:, i * T:(i + 1) * T],
                                     hg[:, i * 128:(i + 1) * 128], cb("w2T"),
                                     start=True, stop=True)
                zg = rt.tile([128, NT * T], FP, tag="zg", name=f"zg_{kc}")
                nc.vector.tensor_tensor(zg[:], z2[:], gt[:], OP.add)
                oh = rt.tile([128, NT * T], FP, tag="oh", name=f"oh_{kc}")
                for i in range(NT):
                    sl = slice(i * T, (i + 1) * T)
                    m8 = rt.tile([128, 8], FP, tag="m8", bufs=3, name=f"m8_{kc}_{i}")
                    nc.vector.max(m8[:], zg[:, sl])
                    nc.vector.tensor_scalar(oh[:, sl], zg[:, sl], m8[:, 0:1],
                                            None, OP.is_equal)
                tp = ps_pre.tile([128, LC], FP, tag="pre", name=f"tp{kc}")
                for i in range(NT):
                    nc.tensor.transpose(tp[0:T, i * 128:(i + 1) * 128],
                                        oh[:, i * T:(i + 1) * T], cf("identF"))
                OT = rt.tile([T, LC], BF, tag="OT", name=f"OT_{kc}")
                nc.scalar.copy(OT[:], tp[0:T, :])

                cpp = ps_pre.tile([128, LC], FP, tag="pre", name=f"cp{kc}")
                mm512(cpp[:], cb("xpC0"), xo0[:], True, False)
                mm512(cpp[:], cb("xpC1"), xo1[:], False, False)
                mm512(cpp[:], cb("PmRep"), OT[:], False, True)
                Crep = C["Crep"] = pj.tile([128, LC], BF, tag="Crep", name=f"Crep_{kc}")
                nc.scalar.copy(Crep[:], cpp[:])
                C["kc"] = kc
                C["hl_pend"] = []
                C["yacc_pend"] = []
                return C


            def emit_yacc(C):
                kc = C["kc"]
                j, sc, ym, rows_, first, last = C["yacc_pend"].pop(0)
                yac = C["yac0"] if j < 16 else C["yac1"]
                mm512(yac[0:rows_, :], sc, ym[:], first, last)
                if j == 15:
                    yD0 = C["yD0"] = yp.tile([128, LC], FPR, tag="yD0",
                                             name=f"yD0_{kc}")
                    nc.vector.scalar_tensor_tensor(
                        yD0[:], C["xs0"][:], cf("Dc0"), C["yac0"][:],
                        OP.mult, OP.add)
                    C["yac1"] = ps_y.tile([64, LC], FP, tag="y",
                                          name=f"y1_{kc}")

            def emit_block(C, j):
                kc = C["kc"]
                if j < 16:
                    dwt = C["dw0"]
                    wa = cr("WdA0")[:, j * P:(j + 1) * P]
                    sc = cb("S80")[:, j * 128:(j + 1) * 128]
                    rows_ = P
                    first, last = j == 0, j == 15
                else:
                    dwt = C["dw1"]
                    wa = cr("WdA1")[:, (j - 16) * P:(j - 15) * P]
                    sc = cb("S81")[:, (j - 16) * 64:(j - 15) * 64]
                    rows_ = 64
                    first, last = j == 16, j == 23
                # deferred hlast copies (Act) — 2 blocks late so Act never
                # stalls waiting for the scan of the current block
                while C["hl_pend"] and C["hl_pend"][0][0] <= j - 2:
                    _, jj, hh = C["hl_pend"].pop(0)
                    nc.scalar.copy(hlast[:, jj:jj + 1], hh[:, LC - 1:LC])
                mmpA = ps_scan.tile([128, LC], FP, tag="scan",
                                    name=f"mmA{kc}_{j}")
                mm512(mmpA[:], wa, dwt[:, 0:LC], True, True)
                dA = bk.tile([P, LC], FP, tag="dA", name=f"dA_{kc}_{j}")
                nc.scalar.activation(dA[:], mmpA[:], AF.Exp)
                mmpB = ps_scan.tile([128, LC], FP, tag="scan",
                                    name=f"mmB{kc}_{j}")
                mm512(mmpB[:], wa, dwt[:, LC:2 * LC], True, True)
                wcp = bk.tile([P, LC], BF, tag="wcp", name=f"wcp_{kc}_{j}")
                nc.scalar.activation(wcp[:], mmpB[:], AF.Copy,
                                     scale=cf("invA"))
                dBu = bk.tile([P, LC], BF, tag="dBu", name=f"dBu_{kc}_{j}")
                nc.vector.tensor_tensor(dBu[:], wcp[:], C["Brep"][:], OP.mult)
                h = hp.tile([P, LC], BF, tag="h", name=f"h_{kc}_{j}")
                init = 0.0 if kc == 0 else hlast[:, j:j + 1]
                nc.vector.tensor_tensor_scan(h[:], dA[:], dBu[:], init,
                                             OP.mult, OP.add)
                if kc < NCH - 1:
                    C["hl_pend"].append((j, j, h))
                ym = hp.tile([P, LC], BF, tag="ym", bufs=4,
                             name=f"ym_{kc}_{j}")
                nc.vector.tensor_tensor(ym[:], h[:], C["Crep"][:], OP.mult)
                # yacc matmuls run 2 blocks late so PE never waits on the
                # DVE scan pipeline mid-stream (HAM throttle avoidance)
                C["yacc_pend"].append((j, sc, ym, rows_, first, last))
                while len(C["yacc_pend"]) > 2:
                    emit_yacc(C)

            def emit_ln(C):
                kc = C["kc"]
                ls = kc * LC
                while C["yacc_pend"]:
                    emit_yacc(C)
                while C["hl_pend"]:
                    _, jj, hh = C["hl_pend"].pop(0)
                    nc.scalar.copy(hlast[:, jj:jj + 1], hh[:, LC - 1:LC])
                yD0 = C["yD0"]
                yD1 = yp.tile([64, LC], FPR, tag="yD1", name=f"yD1_{kc}")
                nc.vector.scalar_tensor_tensor(
                    yD1[:], C["xs1"][:], cf("Dc1"), C["yac1"][:],
                    OP.mult, OP.add)
                ysq0 = yp.tile([128, LC], FPR, tag="ysq0", name=f"ysq0_{kc}")
                nc.scalar.activation(ysq0[:], yD0[:].bitcast(FP), AF.Square)
                ysq1 = yp.tile([64, LC], FPR, tag="ysq1", name=f"ysq1_{kc}")
                nc.scalar.activation(ysq1[:], yD1[:].bitcast(FP), AF.Square)

                s1p = ps_scan.tile([128, LC], FP, tag="scan", name=f"s1{kc}")
                mm512(s1p[0:1, :], cr("onc0"), yD0[:], True, False)
                mm512(s1p[0:1, :], cr("onc1"), yD1[:], False, True)
                s2p = ps_scan.tile([128, LC], FP, tag="scan", name=f"s2{kc}")
                mm512(s2p[0:1, :], cr("onc0"), ysq0[:], True, False)
                mm512(s2p[0:1, :], cr("onc1"), ysq1[:], False, True)
                s1row = rw.tile([1, LC], FP, tag="s1row", name=f"s1row_{kc}")
                s2row = rw.tile([1, LC], FP, tag="s2row", name=f"s2row_{kc}")
                nc.scalar.copy(s1row[:], s1p[0:1, :])
                nc.scalar.copy(s2row[:], s2p[0:1, :])
                nw = LC // 128
                sm = rw.tile([128, 2 * nw], FP, tag="sm", name=f"sm_{kc}")
                nc.sync.dma_start(sm[:, 0:nw], s1row[:])
                nc.sync.dma_start(sm[:, nw:2 * nw], s2row[:])
                mu = rw.tile([128, nw], FP, tag="mu", name=f"mu_{kc}")
                nc.vector.tensor_scalar(mu[:], sm[:, 0:nw], 1.0 / DM, None,
                                        OP.mult)
                musq = rw.tile([128, nw], FP, tag="musq", name=f"musq_{kc}")
                nc.scalar.activation(musq[:], mu[:], AF.Square)
                var = rw.tile([128, nw], FP, tag="var", name=f"var_{kc}")
                nc.vector.scalar_tensor_tensor(
                    var[:], sm[:, nw:2 * nw], 1.0 / DM, musq[:],
                    OP.mult, OP.subtract)
                sd = rw.tile([128, nw], FP, tag="sd", name=f"sd_{kc}")
                nc.scalar.activation(sd[:], var[:], AF.Sqrt, bias=epsc[:])
                inv = rw.tile([128, nw], FP, tag="inv", name=f"inv_{kc}")
                nc.vector.reciprocal(inv[:], sd[:])
                mui = rw.tile([128, nw], FP, tag="mui", name=f"mui_{kc}")
                nc.vector.tensor_tensor(mui[:], mu[:], inv[:], OP.mult)
                irow = rw.tile([1, LC], FP, tag="irow", name=f"irow_{kc}")
                nc.sync.dma_start(irow[:], inv[:])
                mirow = rw.tile([1, LC], FP, tag="mirow", name=f"mirow_{kc}")
                nc.sync.dma_start(mirow[:], mui[:])

                # broadcast g*inv and g*mu*inv via k=1 fp32 matmuls
                ib0 = ps_scan.tile([128, LC], FP, tag="scan", name=f"ib0{kc}")
                mi0 = ps_scan.tile([128, LC], FP, tag="scan", name=f"mi0{kc}")
                mm512(ib0[:], cf("lngr0"), irow[:], True, True)
                mm512(mi0[:], cf("lngr0"), mirow[:], True, True)
                yo0t = yp.tile([128, LC], FP, tag="yo0t", name=f"yo0t_{kc}")
                nc.vector.tensor_tensor(yo0t[:], yD0[:].bitcast(FP), ib0[:],
                                        OP.mult)
                nc.vector.scalar_tensor_tensor(
                    yo0t[:], yo0t[:], cf("lnb0"), mi0[:], OP.add, OP.subtract)
                nc.sync.dma_start(yo0[:, ls:ls + LC], yo0t[:])

                ib1 = ps_scan.tile([64, LC], FP, tag="scan", name=f"ib1{kc}")
                mi1 = ps_scan.tile([64, LC], FP, tag="scan", name=f"mi1{kc}")
                mm512(ib1[:], cf("lngr1"), irow[:], True, True)
                mm512(mi1[:], cf("lngr1"), mirow[:], True, True)
                yo1t = yp.tile([64, LC], FP, tag="yo1t", name=f"yo1t_{kc}")
                nc.vector.tensor_tensor(yo1t[:], yD1[:].bitcast(FP), ib1[:],
                                        OP.mult)
                nc.vector.scalar_tensor_tensor(
                    yo1t[:], yo1t[:], cf("lnb1"), mi1[:], OP.add, OP.subtract)
                nc.sync.dma_start(yo1[:, ls:ls + LC], yo1t[:])

            # ---- software-pipelined chunk loop ----
            Ccur = emit_preamble(0)
            Ccur["yac0"] = ps_y.tile([128, LC], FP, tag="y", name="y0_0")
            for kc in range(NCH):
                for j in range(2):
                    emit_block(Ccur, j)
                Cnext = emit_preamble(kc + 1) if kc + 1 < NCH else None
                for j in range(2, 24):
                    emit_block(Ccur, j)
                if Cnext is not None:
                    Cnext["yac0"] = ps_y.tile([128, LC], FP, tag="y",
                                              name=f"y0_{kc + 1}")
                emit_ln(Ccur)
                Ccur = Cnext

    nc.compile()
    return nc


_PROG = None


def _get_prog():
    global _PROG
    if _PROG is None:
        _PROG = build_program()
    return _PROG


def _make_in_maps(inputs):
    f32 = lambda a: np.ascontiguousarray(np.asarray(a, dtype=np.float32))
    bf16 = lambda a: np.ascontiguousarray(
        np.asarray(np.asarray(a, dtype=np.float32), dtype=ml_dtypes.bfloat16))
    x = {0: f32(inputs["x_rgb"]), 1: f32(inputs["x_e"])}
    u = {0: f32(inputs["u_rgb"]), 1: f32(inputs["u_e"])}
    rw1 = {0: f32(inputs["route_rgb_w1"]), 1: f32(inputs["route_e_w1"])}
    rb1 = {0: f32(inputs["route_rgb_b1"]), 1: f32(inputs["route_e_b1"])}
    rw2 = {0: f32(inputs["route_rgb_w2"]), 1: f32(inputs["route_e_w2"])}
    rb2 = {0: f32(inputs["route_rgb_b2"]), 1: f32(inputs["route_e_b2"])}
    emb = {0: f32(inputs["emb_rgb"]), 1: f32(inputs["emb_e"])}
    tok = {0: f32(inputs["token_rgb_w"]), 1: f32(inputs["token_e_w"])}
    xproj = {0: f32(inputs["xproj_rgb"]), 1: f32(inputs["xproj_e"])}
    dtw = {0: f32(inputs["dtw_rgb"]), 1: f32(inputs["dtw_e"])}
    dtb = {0: f32(inputs["dtb_rgb"]), 1: f32(inputs["dtb_e"])}
    Alog = {0: f32(inputs["Alog_rgb"]), 1: f32(inputs["Alog_e"])}
    Dsk = {0: f32(inputs["D_rgb"]), 1: f32(inputs["D_e"])}
    lng = {0: f32(inputs["ln1_g"]), 1: f32(inputs["ln2_g"])}
    lnb = {0: f32(inputs["ln1_b"]), 1: f32(inputs["ln2_b"])}

    nmap = np.arange(P) % 16   # p -> n
    dmap = np.arange(P) // 16  # p -> d8

    in_maps = []
    for c in range(8):
        b, s = divmod(c, 2)
        o = 1 - s
        xsT = x[s][b].T.copy()          # [192, L]
        xoT = x[o][b].T.copy()
        A = -np.exp(Alog[s])            # [DM, N]
        assert np.allclose(A, A[0:1, :], atol=0), "A must be d-independent"
        Arow = A[0]                     # [N]
        WdA0 = np.zeros((16, 128, P), np.float32)
        for j in range(16):
            WdA0[j, 8 * j + dmap, np.arange(P)] = Arow[nmap]
        WdA1 = np.zeros((8, 64, P), np.float32)
        for j in range(8):
            WdA1[j, 8 * j + dmap, np.arange(P)] = Arow[nmap]
        S80 = np.zeros((16, P, 128), np.float32)
        for j in range(16):
            S80[j, np.arange(P), 8 * j + dmap] = 1.0
        S81 = np.zeros((8, P, 64), np.float32)
        for j in range(8):
            S81[j, np.arange(P), 8 * j + dmap] = 1.0
        Pm = emb[o] @ tok[o]            # [T, N]
        PmRep = np.ascontiguousarray(Pm[:, nmap])                 # [T, P]
        CrepT = np.ascontiguousarray(xproj[o][R + N:R + 2 * N][nmap].T)
        BrepT = np.ascontiguousarray(xproj[s][R:R + N][nmap].T)
        Mdt = (dtw[s] @ xproj[s][:R]).T.copy()                    # [DM, DM]

        cb_consts = {
            "w1T0": rw1[o].T[:128], "w1T1": rw1[o].T[128:], "w2T": rw2[o].T,
            "PmRep": PmRep, "xpC0": CrepT[:128], "xpC1": CrepT[128:],
            "xpB0": BrepT[:128], "xpB1": BrepT[128:],
            "Mdt0": Mdt[:128], "Mdt1": Mdt[128:],
            "S80": np.transpose(S80, (1, 0, 2)).reshape(P, 16 * 128),
            "S81": np.transpose(S81, (1, 0, 2)).reshape(P, 8 * 64),
        }
        cpb_arr = np.zeros((128, CBTOT), np.float32)
        for nm, (off, r, ccols) in CBMAP.items():
            a = np.asarray(cb_consts[nm], np.float32)
            assert a.shape == (r, ccols), (nm, a.shape)
            cpb_arr[:r, off:off + ccols] = a

        cr_consts = {
            "WdA0": np.transpose(WdA0, (1, 0, 2)).reshape(128, 16 * P),
            "WdA1": np.transpose(WdA1, (1, 0, 2)).reshape(64, 8 * P),
            "onc0": np.ones((128, 1), np.float32),
            "onc1": np.ones((64, 1), np.float32),
        }
        cpr_arr = np.zeros((128, CRTOT), np.float32)
        for nm, (off, r, ccols) in CRMAP.items():
            a = np.asarray(cr_consts[nm], np.float32)
            assert a.shape == (r, ccols), (nm, a.shape)
            cpr_arr[:r, off:off + ccols] = a

        cf_consts = {
            "identF": np.eye(128, dtype=np.float32),
            "b1c": rb1[o][:, None], "dtb0": dtb[s][:128, None],
            "dtb1": dtb[s][128:, None],
            "invA": (1.0 / Arow[nmap])[:, None],
            "Dc0": Dsk[s][:128, None], "Dc1": Dsk[s][128:, None],
            "lnb0": lnb[s][:128, None], "lnb1": lnb[s][128:, None],
            "lngr0": lng[s][None, :128], "lngr1": lng[s][None, 128:],
        }
        cpf_arr = np.zeros((128, CFTOT), np.float32)
        for nm, (off, r, ccols) in CFMAP.items():
            a = np.asarray(cf_consts[nm], np.float32)
            assert a.shape == (r, ccols), (nm, a.shape)
            cpf_arr[:r, off:off + ccols] = a

        gqa = (-np.log(-np.log(u[o][b])) + rb2[o][None, :]).astype(np.float32)
        m = {
            "xsT0": bf16(xsT[:128]), "xsT1": bf16(xsT[128:]),
            "xoT0": bf16(xoT[:128]), "xoT1": bf16(xoT[128:]),
            "gq": gqa.reshape(L // 128, 128, T).copy(),
            "cpb": np.ascontiguousarray(cpb_arr.astype(ml_dtypes.bfloat16)),
            "cpr": cpr_arr,
            "cpf": cpf_arr,
        }
        in_maps.append(m)
    return in_maps


def run(inputs, trace=False):
    nc = _get_prog()
    in_maps = _make_in_maps(inputs)
    res = run_bass_kernel_spmd(nc, in_maps, list(range(8)), trace=trace)
    out = np.zeros((2, B, DM, L), np.float32)
    for c in range(8):
        b, s = divmod(c, 2)
        out[s, b, :128] = res.results[c]["yo0"]
        out[s, b, 128:] = res.results[c]["yo1"]
    return out, res


def kernel(**inputs):
    out, _ = run(inputs, trace=False)
    return out


# revision 15
# speedup vs baseline: 1.9993x; 1.0411x over previous
"""Cross-modal selective-scan (ASSM) kernel for 8 TRN2 NeuronCores.

Sharding: one core per (batch, stream) pair: core = b*2 + s, s=0 rgb / s=1 e.
Each core computes the full forward for its stream (routing+gumbel of the
OTHER stream feeds C — cross-modal), the L=4096 selective scan over
(D=192, N=16) states, and the output layernorm. Outputs are gathered on host.

v3 highlights:
 - matmuls in bf16 (x-projections / routing / yacc) and fp32r (scan args),
   never plain fp32 on big streams -> ~4x PE throughput per column.
 - the state-injection/readout chain (dBu, h, ym, B, C) runs in bf16: DVE
   2x mode for the elementwise ops, half the SBUF traffic. The compounding
   decay dA stays fp32 (bf16 decay quantization accumulates over the scan).
 - gumbel noise -ln(-ln u) (+ b2 routing bias) precomputed on host.
 - the next chunk's load/routing/projection preamble is emitted in the
   middle of the current chunk's scan phase (software pipelining).
 - GpSimd handles SBUF-only side ops (w-build, softplus add, one-hot eq,
   y^2); it cannot access PSUM, so PSUM readers stay on Vector/Scalar.
"""

import numpy as np
import ml_dtypes

import concourse.bass as bass
import concourse.bacc as bacc
import concourse.mybir as mybir
import concourse.tile as tile
from concourse.bass_utils import run_bass_kernel_spmd

FP = mybir.dt.float32
FPR = mybir.dt.float32r
BF = mybir.dt.bfloat16
OP = mybir.AluOpType
AF = mybir.ActivationFunctionType

B, L, DM, N, R, T, H3 = 4, 4096, 192, 16, 12, 64, 64
P = 128
LC = 1024               # chunk along L
NCH = L // LC           # 4
SUB = 512
LEPS = 1e-5
NT = LC // 128          # 8 token tiles per chunk

# ---- packed-constant layouts: name -> (col offset, rows, cols) ----
def _pack(shapes):
    m, off = {}, 0
    for nm, r, c in shapes:
        m[nm] = (off, r, c)
        off += c
    return m, off

_CB_SHAPES = [
    ("w1T0", 128, 64), ("w1T1", 64, 64), ("w2T", 64, 64), ("PmRep", 64, 128),
    ("xpC0", 128, 128), ("xpC1", 64, 128), ("xpB0", 128, 128),
    ("xpB1", 64, 128), ("Mdt0", 128, 192), ("Mdt1", 64, 192),
    ("S80", 128, 16 * 128), ("S81", 128, 8 * 64),
    ("WdA0", 128, 16 * 128), ("WdA1", 64, 8 * 128),
]
CBMAP, CBTOT = _pack(_CB_SHAPES)

_CR_SHAPES = [
    ("onc0", 128, 1), ("onc1", 64, 1), ("lngr0", 1, 128), ("lngr1", 1, 64),
]
CRMAP, CRTOT = _pack(_CR_SHAPES)

_CF_SHAPES = [
    ("identF", 128, 128), ("b1c", 64, 1), ("dtb0", 128, 1), ("dtb1", 64, 1),
    ("invA", 128, 1), ("Dc0", 128, 1), ("Dc1", 64, 1), ("lnb0", 128, 1),
    ("lnb1", 64, 1),
]
CFMAP, CFTOT = _pack(_CF_SHAPES)


def build_program():
    nc = bacc.Bacc("TRN2", target_bir_lowering=False, debug=False)

    xsT0 = nc.declare_dram_parameter("xsT0", [128, L], BF, isOutput=False)
    xsT1 = nc.declare_dram_parameter("xsT1", [64, L], BF, isOutput=False)
    xoT0 = nc.declare_dram_parameter("xoT0", [128, L], BF, isOutput=False)
    xoT1 = nc.declare_dram_parameter("xoT1", [64, L], BF, isOutput=False)
    gq = nc.declare_dram_parameter("gq", [L // 128, 128, T], FP, isOutput=False)
    cpb = nc.declare_dram_parameter("cpb", [128, CBTOT], BF, isOutput=False)
    cpr = nc.declare_dram_parameter("cpr", [128, CRTOT], FPR, isOutput=False)
    cpf = nc.declare_dram_parameter("cpf", [128, CFTOT], FP, isOutput=False)
    yo0 = nc.declare_dram_parameter("yo0", [128, L], FP, isOutput=True)
    yo1 = nc.declare_dram_parameter("yo1", [64, L], FP, isOutput=True)

    with tile.TileContext(nc) as tc:
        with (
            tc.tile_pool(name="const", bufs=1) as cp,
            tc.tile_pool(name="xin", bufs=2) as xp,
            tc.tile_pool(name="dwp", bufs=2) as dwp,
            tc.tile_pool(name="proj", bufs=2) as pj,
            tc.tile_pool(name="route", bufs=2) as rt,
            tc.tile_pool(name="spool", bufs=2) as sp_,
            tc.tile_pool(name="blk", bufs=3) as bk,
            tc.tile_pool(name="hpool", bufs=3) as hp,
            tc.tile_pool(name="ypool", bufs=1) as yp,
            tc.tile_pool(name="rows", bufs=1) as rw,
            tc.tile_pool(name="persist", bufs=1) as pe_,
            tc.tile_pool(name="ps_scan", bufs=2, space="PSUM") as ps_scan,
            tc.tile_pool(name="ps_pre", bufs=1, space="PSUM") as ps_pre,
            tc.tile_pool(name="ps_y", bufs=1, space="PSUM") as ps_y,
        ):
            cbk = cp.tile([128, CBTOT], BF, tag="cpb")
            nc.sync.dma_start(cbk[:], cpb[:])
            crk = cp.tile([128, CRTOT], FPR, tag="cpr")
            nc.sync.dma_start(crk[:], cpr[:])
            cfk = cp.tile([128, CFTOT], FP, tag="cpf")
            nc.sync.dma_start(cfk[:], cpf[:])

            def cb(name):
                off, r, c = CBMAP[name]
                return cbk[0:r, off:off + c]

            def cr(name):
                off, r, c = CRMAP[name]
                return crk[0:r, off:off + c]

            def cf(name):
                off, r, c = CFMAP[name]
                return cfk[0:r, off:off + c]

            def mm512(out, lhsT, rhs, start, stop):
                # ISA caps the moving operand at 512 elements per matmul
                n = rhs.shape[-1]
                for q in range(0, n, 512):
                    e = min(q + 512, n)
                    nc.tensor.matmul(out[:, q:e], lhsT, rhs[:, q:e],
                                     start=start, stop=stop)

            hlast = pe_.tile([P, 24], FP)
            epsc = pe_.tile([128, 1], FP)
            nc.vector.memset(epsc[:], LEPS)

            def emit_preamble(kc):
                """Loads + projections + routing + w-build for chunk kc."""
                ls = kc * LC
                c0 = ls // 128
                C = {}
                xs0 = C["xs0"] = xp.tile([128, LC], BF, tag="xs0", name=f"xs0_{kc}")
                xs1 = C["xs1"] = xp.tile([64, LC], BF, tag="xs1", name=f"xs1_{kc}")
                xo0 = xp.tile([128, LC], BF, tag="xo0", name=f"xo0_{kc}")
                xo1 = xp.tile([64, LC], BF, tag="xo1", name=f"xo1_{kc}")
                gt = xp.tile([128, NT * T], FP, tag="gt", name=f"gt_{kc}")
                nc.sync.dma_start(xs0[:], xsT0[:, ls:ls + LC])
                nc.sync.dma_start(xs1[:], xsT1[:, ls:ls + LC])
                nc.sync.dma_start(xo0[:], xoT0[:, ls:ls + LC])
                nc.sync.dma_start(xo1[:], xoT1[:, ls:ls + LC])
                nc.sync.dma_start(
                    gt[:].rearrange("p (c t) -> p c t", c=NT),
                    gq[c0:c0 + NT].rearrange("c p t -> p c t"))

                # dt -> softplus -> dl ; w = dl*x   (dw = [dl | w])
                dw0 = C["dw0"] = dwp.tile([128, 2 * LC], BF, tag="dw0", name=f"dw0_{kc}")
                dw1 = C["dw1"] = dwp.tile([64, 2 * LC], BF, tag="dw1", name=f"dw1_{kc}")
                dtp0 = ps_pre.tile([128, LC], FP, tag="pre", name=f"dt0{kc}")
                mm512(dtp0[:], cb("Mdt0")[:, 0:128], xs0[:], True, False)
                mm512(dtp0[:], cb("Mdt1")[:, 0:128], xs1[:], False, True)
                # softplus(x) = ln(exp(x) + 1); x = dt + dtb stays < ~3 here
                sp0 = sp_.tile([128, LC], FP, tag="sp0", name=f"sp0_{kc}")
                nc.scalar.activation(sp0[:], dtp0[:], AF.Exp, bias=cf("dtb0"))
                nc.scalar.activation(dw0[:, 0:LC], sp0[:], AF.Ln, bias=1.0)
                dtp1 = ps_pre.tile([64, LC], FP, tag="pre", name=f"dt1{kc}")
                mm512(dtp1[:], cb("Mdt0")[:, 128:DM], xs0[:], True, False)
                mm512(dtp1[:], cb("Mdt1")[:, 128:DM], xs1[:], False, True)
                sp1 = sp_.tile([64, LC], FP, tag="sp1", name=f"sp1_{kc}")
                nc.scalar.activation(sp1[:], dtp1[:], AF.Exp, bias=cf("dtb1"))
                nc.scalar.activation(dw1[:, 0:LC], sp1[:], AF.Ln, bias=1.0)
                nc.gpsimd.tensor_tensor(dw0[:, LC:2 * LC], dw0[:, 0:LC],
                                        xs0[:], OP.mult)
                nc.gpsimd.tensor_tensor(dw1[:, LC:2 * LC], dw1[:, 0:LC],
                                        xs1[:], OP.mult)

                bp = ps_pre.tile([128, LC], FP, tag="pre", name=f"bp{kc}")
                mm512(bp[:], cb("xpB0"), xs0[:], True, False)
                mm512(bp[:], cb("xpB1"), xs1[:], False, True)
                Brep = C["Brep"] = pj.tile([128, LC], BF, tag="Brep", name=f"Brep_{kc}")
                nc.scalar.copy(Brep[:], bp[:])

                # routing of the other stream -> one-hot OT
                zt = ps_pre.tile([128, LC], FP, tag="pre", name=f"zt{kc}")
                mm512(zt[0:H3, :], cb("w1T0"), xo0[:], True, False)
                mm512(zt[0:H3, :], cb("w1T1"), xo1[:], False, True)
                hg = rt.tile([H3, LC], BF, tag="hg", name=f"hg_{kc}")
                nc.scalar.activation(hg[:], zt[0:H3, :], AF.Gelu,
                                     bias=cf("b1c"))
                z2 = ps_scan.tile([128, NT * T], FP, tag="scan", name=f"z2{kc}")
                for i in range(NT):
                    nc.tensor.matmul(z2[:, i * T:(i + 1) * T],
                                     hg[:, i * 128:(i + 1) * 128], cb("w2T"),
                                     start=True, stop=True)
                zg = rt.tile([128, NT * T], FP, tag="zg", name=f"zg_{kc}")
                nc.vector.tensor_tensor(zg[:], z2[:], gt[:], OP.add)
                oh = rt.tile([128, NT * T], FP, tag="oh", name=f"oh_{kc}")
                for i in range(NT):
                    sl = slice(i * T, (i + 1) * T)
                    m8 = rt.tile([128, 8], FP, tag="m8", bufs=3, name=f"m8_{kc}_{i}")
                    nc.vector.max(m8[:], zg[:, sl])
                    nc.vector.tensor_scalar(oh[:, sl], zg[:, sl], m8[:, 0:1],
                                            None, OP.is_equal)
                tp = ps_pre.tile([128, LC], FP, tag="pre", name=f"tp{kc}")
                for i in range(NT):
                    nc.tensor.transpose(tp[0:T, i * 128:(i + 1) * 128],
                                        oh[:, i * T:(i + 1) * T], cf("identF"))
                OT = rt.tile([T, LC], BF, tag="OT", name=f"OT_{kc}")
                nc.scalar.copy(OT[:], tp[0:T, :])

                cpp = ps_pre.tile([128, LC], FP, tag="pre", name=f"cp{kc}")
                mm512(cpp[:], cb("xpC0"), xo0[:], True, False)
                mm512(cpp[:], cb("xpC1"), xo1[:], False, False)
                mm512(cpp[:], cb("PmRep"), OT[:], False, True)
                Crep = C["Crep"] = pj.tile([128, LC], BF, tag="Crep", name=f"Crep_{kc}")
                nc.scalar.copy(Crep[:], cpp[:])
                C["kc"] = kc
                C["hl_pend"] = []
                C["yacc_pend"] = []
                return C


            def emit_yacc(C):
                kc = C["kc"]
                j, sc, ym, rows_, first, last = C["yacc_pend"].pop(0)
                yac = C["yac0"] if j < 16 else C["yac1"]
                mm512(yac[0:rows_, :], sc, ym[:], first, last)
                if j == 15:
                    yD0 = C["yD0"] = yp.tile([128, LC], FPR, tag="yD0",
                                             name=f"yD0_{kc}")
                    nc.vector.scalar_tensor_tensor(
                        yD0[:], C["xs0"][:], cf("Dc0"), C["yac0"][:],
                        OP.mult, OP.add)
                    C["yac1"] = ps_y.tile([64, LC], FP, tag="y",
                                          name=f"y1_{kc}")

            def emit_block(C, j):
                kc = C["kc"]
                if j < 16:
                    dwt = C["dw0"]
                    wa = cb("WdA0")[:, j * P:(j + 1) * P]
                    sc = cb("S80")[:, j * 128:(j + 1) * 128]
                    rows_ = P
                    first, last = j == 0, j == 15
                else:
                    dwt = C["dw1"]
                    wa = cb("WdA1")[:, (j - 16) * P:(j - 15) * P]
                    sc = cb("S81")[:, (j - 16) * 64:(j - 15) * 64]
                    rows_ = 64
                    first, last = j == 16, j == 23
                # deferred hlast copies (Act) — 2 blocks late so Act never
                # stalls waiting for the scan of the current block
                while C["hl_pend"] and C["hl_pend"][0][0] <= j - 2:
                    _, jj, hh = C["hl_pend"].pop(0)
                    nc.scalar.copy(hlast[:, jj:jj + 1], hh[:, LC - 1:LC])
                mmpA = ps_scan.tile([128, LC], FP, tag="scan",
                                    name=f"mmA{kc}_{j}")
                mm512(mmpA[:], wa, dwt[:, 0:LC], True, True)
                dA = bk.tile([P, LC], FP, tag="dA", name=f"dA_{kc}_{j}")
                nc.scalar.activation(dA[:], mmpA[:], AF.Exp)
                mmpB = ps_scan.tile([128, LC], FP, tag="scan",
                                    name=f"mmB{kc}_{j}")
                mm512(mmpB[:], wa, dwt[:, LC:2 * LC], True, True)
                wcp = bk.tile([P, LC], BF, tag="wcp", name=f"wcp_{kc}_{j}")
                nc.scalar.activation(wcp[:], mmpB[:], AF.Copy,
                                     scale=cf("invA"))
                dBu = bk.tile([P, LC], BF, tag="dBu", name=f"dBu_{kc}_{j}")
                nc.vector.tensor_tensor(dBu[:], wcp[:], C["Brep"][:], OP.mult)
                h = hp.tile([P, LC], BF, tag="h", name=f"h_{kc}_{j}")
                init = 0.0 if kc == 0 else hlast[:, j:j + 1]
                nc.vector.tensor_tensor_scan(h[:], dA[:], dBu[:], init,
                                             OP.mult, OP.add)
                if kc < NCH - 1:
                    C["hl_pend"].append((j, j, h))
                ym = hp.tile([P, LC], BF, tag="ym", bufs=4,
                             name=f"ym_{kc}_{j}")
                nc.vector.tensor_tensor(ym[:], h[:], C["Crep"][:], OP.mult)
                # yacc matmuls run 2 blocks late so PE never waits on the
                # DVE scan pipeline mid-stream (HAM throttle avoidance)
                C["yacc_pend"].append((j, sc, ym, rows_, first, last))
                while len(C["yacc_pend"]) > 2:
                    emit_yacc(C)

            def emit_ln(C):
                kc = C["kc"]
                ls = kc * LC
                while C["yacc_pend"]:
                    emit_yacc(C)
                while C["hl_pend"]:
                    _, jj, hh = C["hl_pend"].pop(0)
                    nc.scalar.copy(hlast[:, jj:jj + 1], hh[:, LC - 1:LC])
                yD0 = C["yD0"]
                yD1 = yp.tile([64, LC], FPR, tag="yD1", name=f"yD1_{kc}")
                nc.vector.scalar_tensor_tensor(
                    yD1[:], C["xs1"][:], cf("Dc1"), C["yac1"][:],
                    OP.mult, OP.add)
                ysq0 = yp.tile([128, LC], FPR, tag="ysq0", name=f"ysq0_{kc}")
                nc.scalar.activation(ysq0[:], yD0[:].bitcast(FP), AF.Square)
                ysq1 = yp.tile([64, LC], FPR, tag="ysq1", name=f"ysq1_{kc}")
                nc.scalar.activation(ysq1[:], yD1[:].bitcast(FP), AF.Square)

                s1p = ps_scan.tile([128, LC], FP, tag="scan", name=f"s1{kc}")
                mm512(s1p[0:1, :], cr("onc0"), yD0[:], True, False)
                mm512(s1p[0:1, :], cr("onc1"), yD1[:], False, True)
                s2p = ps_scan.tile([128, LC], FP, tag="scan", name=f"s2{kc}")
                mm512(s2p[0:1, :], cr("onc0"), ysq0[:], True, False)
                mm512(s2p[0:1, :], cr("onc1"), ysq1[:], False, True)
                s1row = rw.tile([1, LC], FP, tag="s1row", name=f"s1row_{kc}")
                s2row = rw.tile([1, LC], FP, tag="s2row", name=f"s2row_{kc}")
                nc.scalar.copy(s1row[:], s1p[0:1, :])
                nc.scalar.copy(s2row[:], s2p[0:1, :])
                # stats directly on the [1, LC] row layout
                murow = rw.tile([1, LC], FP, tag="murow", name=f"murow_{kc}")
                nc.vector.tensor_scalar(murow[:], s1row[:], 1.0 / DM, None,
                                        OP.mult)
                msqr = rw.tile([1, LC], FP, tag="msqr", name=f"msqr_{kc}")
                nc.scalar.activation(msqr[:], murow[:], AF.Square)
                varr = rw.tile([1, LC], FP, tag="varr", name=f"varr_{kc}")
                nc.vector.scalar_tensor_tensor(
                    varr[:], s2row[:], 1.0 / DM, msqr[:],
                    OP.mult, OP.subtract)
                sdr = rw.tile([1, LC], FP, tag="sdr", name=f"sdr_{kc}")
                nc.scalar.activation(sdr[:], varr[:], AF.Sqrt,
                                     bias=epsc[0:1, :])
                irow = rw.tile([1, LC], FPR, tag="irow", name=f"irow_{kc}")
                mirow = rw.tile([1, LC], FPR, tag="mirow", name=f"mirow_{kc}")
                with nc.allow_low_precision(reason="fp32r rows for broadcast"):
                    nc.vector.reciprocal(irow[:], sdr[:])
                    nc.vector.tensor_tensor(mirow[:], murow[:],
                                            irow[:].bitcast(FP), OP.mult)

                # broadcast g*inv and g*mu*inv via k=1 fp32 matmuls
                ib0 = ps_scan.tile([128, LC], FP, tag="scan", name=f"ib0{kc}")
                mi0 = ps_scan.tile([128, LC], FP, tag="scan", name=f"mi0{kc}")
                mm512(ib0[:], cr("lngr0"), irow[:], True, True)
                mm512(mi0[:], cr("lngr0"), mirow[:], True, True)
                yo0t = yp.tile([128, LC], FP, tag="yo0t", name=f"yo0t_{kc}")
                nc.vector.tensor_tensor(yo0t[:], yD0[:].bitcast(FP), ib0[:],
                                        OP.mult)
                nc.vector.scalar_tensor_tensor(
                    yo0t[:], yo0t[:], cf("lnb0"), mi0[:], OP.add, OP.subtract)
                nc.sync.dma_start(yo0[:, ls:ls + LC], yo0t[:])

                ib1 = ps_scan.tile([64, LC], FP, tag="scan", name=f"ib1{kc}")
                mi1 = ps_scan.tile([64, LC], FP, tag="scan", name=f"mi1{kc}")
                mm512(ib1[:], cr("lngr1"), irow[:], True, True)
                mm512(mi1[:], cr("lngr1"), mirow[:], True, True)
                yo1t = yp.tile([64, LC], FP, tag="yo1t", name=f"yo1t_{kc}")
                nc.vector.tensor_tensor(yo1t[:], yD1[:].bitcast(FP), ib1[:],
                                        OP.mult)
                nc.vector.scalar_tensor_tensor(
                    yo1t[:], yo1t[:], cf("lnb1"), mi1[:], OP.add, OP.subtract)
                nc.sync.dma_start(yo1[:, ls:ls + LC], yo1t[:])

            # ---- software-pipelined chunk loop ----
            Ccur = emit_preamble(0)
            Ccur["yac0"] = ps_y.tile([128, LC], FP, tag="y", name="y0_0")
            for kc in range(NCH):
                for j in range(2):
                    emit_block(Ccur, j)
                Cnext = emit_preamble(kc + 1) if kc + 1 < NCH else None
                for j in range(2, 24):
                    emit_block(Ccur, j)
                if Cnext is not None:
                    Cnext["yac0"] = ps_y.tile([128, LC], FP, tag="y",
                                              name=f"y0_{kc + 1}")
                emit_ln(Ccur)
                Ccur = Cnext

    nc.compile()
    return nc


_PROG = None


def _get_prog():
    global _PROG
    if _PROG is None:
        _PROG = build_program()
    return _PROG


def _make_in_maps(inputs):
    f32 = lambda a: np.ascontiguousarray(np.asarray(a, dtype=np.float32))
    bf16 = lambda a: np.ascontiguousarray(
        np.asarray(np.asarray(a, dtype=np.float32), dtype=ml_dtypes.bfloat16))
    x = {0: f32(inputs["x_rgb"]), 1: f32(inputs["x_e"])}
    u = {0: f32(inputs["u_rgb"]), 1: f32(inputs["u_e"])}
    rw1 = {0: f32(inputs["route_rgb_w1"]), 1: f32(inputs["route_e_w1"])}
    rb1 = {0: f32(inputs["route_rgb_b1"]), 1: f32(inputs["route_e_b1"])}
    rw2 = {0: f32(inputs["route_rgb_w2"]), 1: f32(inputs["route_e_w2"])}
    rb2 = {0: f32(inputs["route_rgb_b2"]), 1: f32(inputs["route_e_b2"])}
    emb = {0: f32(inputs["emb_rgb"]), 1: f32(inputs["emb_e"])}
    tok = {0: f32(inputs["token_rgb_w"]), 1: f32(inputs["token_e_w"])}
    xproj = {0: f32(inputs["xproj_rgb"]), 1: f32(inputs["xproj_e"])}
    dtw = {0: f32(inputs["dtw_rgb"]), 1: f32(inputs["dtw_e"])}
    dtb = {0: f32(inputs["dtb_rgb"]), 1: f32(inputs["dtb_e"])}
    Alog = {0: f32(inputs["Alog_rgb"]), 1: f32(inputs["Alog_e"])}
    Dsk = {0: f32(inputs["D_rgb"]), 1: f32(inputs["D_e"])}
    lng = {0: f32(inputs["ln1_g"]), 1: f32(inputs["ln2_g"])}
    lnb = {0: f32(inputs["ln1_b"]), 1: f32(inputs["ln2_b"])}

    nmap = np.arange(P) % 16   # p -> n
    dmap = np.arange(P) // 16  # p -> d8

    in_maps = []
    for c in range(8):
        b, s = divmod(c, 2)
        o = 1 - s
        xsT = x[s][b].T.copy()          # [192, L]
        xoT = x[o][b].T.copy()
        A = -np.exp(Alog[s])            # [DM, N]
        assert np.allclose(A, A[0:1, :], atol=0), "A must be d-independent"
        Arow = A[0]                     # [N]
        WdA0 = np.zeros((16, 128, P), np.float32)
        for j in range(16):
            WdA0[j, 8 * j + dmap, np.arange(P)] = Arow[nmap]
        WdA1 = np.zeros((8, 64, P), np.float32)
        for j in range(8):
            WdA1[j, 8 * j + dmap, np.arange(P)] = Arow[nmap]
        S80 = np.zeros((16, P, 128), np.float32)
        for j in range(16):
            S80[j, np.arange(P), 8 * j + dmap] = 1.0
        S81 = np.zeros((8, P, 64), np.float32)
        for j in range(8):
            S81[j, np.arange(P), 8 * j + dmap] = 1.0
        Pm = emb[o] @ tok[o]            # [T, N]
        PmRep = np.ascontiguousarray(Pm[:, nmap])                 # [T, P]
        CrepT = np.ascontiguousarray(xproj[o][R + N:R + 2 * N][nmap].T)
        BrepT = np.ascontiguousarray(xproj[s][R:R + N][nmap].T)
        Mdt = (dtw[s] @ xproj[s][:R]).T.copy()                    # [DM, DM]

        cb_consts = {
            "w1T0": rw1[o].T[:128], "w1T1": rw1[o].T[128:], "w2T": rw2[o].T,
            "PmRep": PmRep, "xpC0": CrepT[:128], "xpC1": CrepT[128:],
            "xpB0": BrepT[:128], "xpB1": BrepT[128:],
            "Mdt0": Mdt[:128], "Mdt1": Mdt[128:],
            "S80": np.transpose(S80, (1, 0, 2)).reshape(P, 16 * 128),
            "S81": np.transpose(S81, (1, 0, 2)).reshape(P, 8 * 64),
            "WdA0": np.transpose(WdA0, (1, 0, 2)).reshape(128, 16 * P),
            "WdA1": np.transpose(WdA1, (1, 0, 2)).reshape(64, 8 * P),
        }
        cpb_arr = np.zeros((128, CBTOT), np.float32)
        for nm, (off, r, ccols) in CBMAP.items():
            a = np.asarray(cb_consts[nm], np.float32)
            assert a.shape == (r, ccols), (nm, a.shape)
            cpb_arr[:r, off:off + ccols] = a

        cr_consts = {
            "onc0": np.ones((128, 1), np.float32),
            "onc1": np.ones((64, 1), np.float32),
            "lngr0": lng[s][None, :128], "lngr1": lng[s][None, 128:],
        }
        cpr_arr = np.zeros((128, CRTOT), np.float32)
        for nm, (off, r, ccols) in CRMAP.items():
            a = np.asarray(cr_consts[nm], np.float32)
            assert a.shape == (r, ccols), (nm, a.shape)
            cpr_arr[:r, off:off + ccols] = a

        cf_consts = {
            "identF": np.eye(128, dtype=np.float32),
            "b1c": rb1[o][:, None], "dtb0": dtb[s][:128, None],
            "dtb1": dtb[s][128:, None],
            "invA": (1.0 / Arow[nmap])[:, None],
            "Dc0": Dsk[s][:128, None], "Dc1": Dsk[s][128:, None],
            "lnb0": lnb[s][:128, None], "lnb1": lnb[s][128:, None],
        }
        cpf_arr = np.zeros((128, CFTOT), np.float32)
        for nm, (off, r, ccols) in CFMAP.items():
            a = np.asarray(cf_consts[nm], np.float32)
            assert a.shape == (r, ccols), (nm, a.shape)
            cpf_arr[:r, off:off + ccols] = a

        gqa = (-np.log(-np.log(u[o][b])) + rb2[o][None, :]).astype(np.float32)
        m = {
            "xsT0": bf16(xsT[:128]), "xsT1": bf16(xsT[128:]),
            "xoT0": bf16(xoT[:128]), "xoT1": bf16(xoT[128:]),
            "gq": gqa.reshape(L // 128, 128, T).copy(),
            "cpb": np.ascontiguousarray(cpb_arr.astype(ml_dtypes.bfloat16)),
            "cpr": cpr_arr,
            "cpf": cpf_arr,
        }
        in_maps.append(m)
    return in_maps


def run(inputs, trace=False):
    nc = _get_prog()
    in_maps = _make_in_maps(inputs)
    res = run_bass_kernel_spmd(nc, in_maps, list(range(8)), trace=trace)
    out = np.zeros((2, B, DM, L), np.float32)
    for c in range(8):
        b, s = divmod(c, 2)
        out[s, b, :128] = res.results[c]["yo0"]
        out[s, b, 128:] = res.results[c]["yo1"]
    return out, res


def kernel(**inputs):
    out, _ = run(inputs, trace=False)
    return out
